# revision 1
# baseline (speedup 1.0000x reference)
"""DGCNN (4x GCNConv + sort-pool + MLP) on 8 trn2 NeuronCores.

Strategy: graph-parallel sharding (ranks 0-3: 13 graphs, 4-7: 12 graphs).
Per layer: u = dinv*h staged node-major -> AllGather full table ->
indirect-DMA gather of in-edge rows (degree-sorted rounds of 128 dsts)
accumulated directly in PSUM via identity-stationary matmuls -> dinv scale
-> feature transform on PE -> tanh. Sort-pool via max8/max_index/
match_replace, pooled rows extracted with ap_gather, classifier on PE.
"""
import os
import numpy as np

N = 50000
G = 100
NPG = 500
E = 800000
F = 64
K_TOP = 15
CAT = 193
NCORES = 8
SHARD = 6656          # padded rows per rank in the AllGather table
NTOT = SHARD * NCORES
NROUND = SHARD // 128  # 52
BN_EPS = 1e-5

GRAPHS_PER_CORE = [13, 13, 13, 13, 12, 12, 12, 12]
GSTART = np.concatenate([[0], np.cumsum(GRAPHS_PER_CORE)])  # [0,13,...,100]

_CACHE = {}


def _prep(x, edge_index):
    """Host-side sharding/index preprocessing. Pure numpy."""
    src = edge_index[0].astype(np.int64)
    dst = edge_index[1].astype(np.int64)

    deg = np.bincount(dst, minlength=N).astype(np.float32) + np.float32(1.0)
    dinv = deg ** np.float32(-0.5)
    indeg = np.bincount(dst, minlength=N).astype(np.int64)

    node_graph = np.arange(N) // NPG
    node_rank = np.searchsorted(GSTART, node_graph, side="right") - 1  # [N]

    # per-core local pi order: graphs contiguous (500 slots each at the
    # front), nodes within a graph sorted by in-degree descending.
    order_in_graph = np.zeros(N, np.int64)
    for g in range(G):
        lo = g * NPG
        d = indeg[lo:lo + NPG]
        o = np.argsort(-d, kind="stable")
        r = np.empty(NPG, np.int64)
        r[o] = np.arange(NPG)
        order_in_graph[lo:lo + NPG] = r
    # slot of node within its core
    slot_of = (node_graph - GSTART[node_rank]) * NPG + order_in_graph
    pidx = node_rank * SHARD + slot_of  # padded global table row of each node

    # per-core edge lists grouped by dst slot
    e_rank = node_rank[dst]
    counts_all = np.zeros((NCORES, SHARD), np.int64)
    per_core = []
    for c in range(NCORES):
        m = e_rank == c
        s_p = pidx[src[m]].astype(np.int64)     # src padded row
        d_slot = slot_of[dst[m]]                # dst local slot
        o = np.argsort(d_slot, kind="stable")
        s_p = s_p[o]
        d_slot = d_slot[o]
        cnt = np.bincount(d_slot, minlength=SHARD)
        counts_all[c] = cnt
        per_core.append((s_p, d_slot, cnt))

    # uniform per-round slab depth across cores
    D = counts_all.reshape(NCORES, NROUND, 128).max(axis=(0, 2))  # [NROUND]
    callbase = np.concatenate([[0], np.cumsum(D)])
    CALLS = int(callbase[-1])

    # build per-core gidx [128, CALLS] int32 + node-major x + dinv
    gidxs, x_nms, dinv_nms = [], [], []
    for c in range(NCORES):
        s_p, d_slot, cnt = per_core[c]
        off = np.concatenate([[0], np.cumsum(cnt)])
        zrow = c * SHARD + (SHARD - 1)
        gid = np.full((128, CALLS), zrow, np.int32)
        j_within = np.arange(len(d_slot)) - off[d_slot]
        k = d_slot // 128
        p = d_slot % 128
        call = callbase[k] + j_within
        gid[p, call] = s_p.astype(np.int32)
        gidxs.append(gid)

        # node-major x in pi order [128, 52, 64]; pads zero
        xs = np.zeros((SHARD, F), np.float32)
        nodes = np.arange(NPG * GSTART[c] * 1, NPG * GSTART[c + 1])
        xs[slot_of[nodes]] = x[nodes]
        x_nms.append(np.ascontiguousarray(
            xs.reshape(NROUND, 128, F).transpose(1, 0, 2)))

        dv = np.zeros(SHARD, np.float32)
        dv[slot_of[nodes]] = dinv[nodes]
        dinv_nms.append(np.ascontiguousarray(dv.reshape(NROUND, 128).T))

    return CALLS, D, callbase, gidxs, x_nms, dinv_nms


def _pack_classifier(inp):
    """Reshape classifier weights for the device layout."""
    Wc0 = np.asarray(inp["Wc0"], np.float32)           # [2895, 256]
    Wc0r = Wc0.reshape(K_TOP, CAT, 256)
    WA = np.ascontiguousarray(Wc0r[:, 0:128, :].transpose(1, 0, 2))   # [128,15,256]
    WB = np.zeros((80, K_TOP, 256), np.float32)
    WB[0:65] = Wc0r[:, 128:193, :].transpose(1, 0, 2)
    sc_full = np.asarray(inp["gamma"], np.float32) * np.float32(
        1.0 / np.sqrt(1.0 + BN_EPS))
    be_full = (np.asarray(inp["beta"], np.float32)
               + np.asarray(inp["bc0"], np.float32) * sc_full)
    sc = np.ascontiguousarray(sc_full.reshape(2, 128).T)  # [128, 2]
    be = np.ascontiguousarray(be_full.reshape(2, 128).T)
    Wc1 = np.asarray(inp["Wc1"], np.float32)              # [256, 128]
    Wc1s = np.ascontiguousarray(Wc1.reshape(2, 128, 128).transpose(1, 0, 2))
    return {
        "WA": WA, "WB": WB, "sc": sc, "be": be, "Wc1s": Wc1s,
        "bc1": np.asarray(inp["bc1"], np.float32).reshape(128, 1),
        "Wc2": np.asarray(inp["Wc2"], np.float32),
        "bc2": np.asarray(inp["bc2"], np.float32).reshape(64, 1),
        "Wc3": np.asarray(inp["Wc3"], np.float32),
        "bc3": np.asarray(inp["bc3"], np.float32).reshape(1, 1),
    }


def _build(CALLS, D, callbase):
    import concourse.bass as bass
    import concourse.bacc as bacc
    import concourse.mybir as mybir
    from concourse import tile
    from concourse.masks import make_identity

    f32 = mybir.dt.float32
    AF = mybir.ActivationFunctionType

    nc = bacc.Bacc("TRN2", target_bir_lowering=False, debug=False,
                   num_devices=NCORES)

    # ---- I/O ----
    x_nm_d = nc.dram_tensor("x_nm", [128, NROUND, F], f32, kind="ExternalInput")
    gidx_d = nc.dram_tensor("gidx", [128, CALLS], mybir.dt.int32, kind="ExternalInput")
    dinv_d = nc.dram_tensor("dinv_nm", [128, NROUND], f32, kind="ExternalInput")
    W_d = [nc.dram_tensor(f"W{i}", [F, F if i < 3 else 1], f32, kind="ExternalInput") for i in range(4)]
    b_d = [nc.dram_tensor(f"b{i}", [F if i < 3 else 1, 1], f32, kind="ExternalInput") for i in range(4)]
    WA_d = nc.dram_tensor("WA", [128, K_TOP, 256], f32, kind="ExternalInput")
    WB_d = nc.dram_tensor("WB", [80, K_TOP, 256], f32, kind="ExternalInput")
    sc_d = nc.dram_tensor("sc", [128, 2], f32, kind="ExternalInput")
    be_d = nc.dram_tensor("be", [128, 2], f32, kind="ExternalInput")
    Wc1_d = nc.dram_tensor("Wc1s", [128, 2, 128], f32, kind="ExternalInput")
    bc1_d = nc.dram_tensor("bc1", [128, 1], f32, kind="ExternalInput")
    Wc2_d = nc.dram_tensor("Wc2", [128, F], f32, kind="ExternalInput")
    bc2_d = nc.dram_tensor("bc2", [F, 1], f32, kind="ExternalInput")
    Wc3_d = nc.dram_tensor("Wc3", [F, 1], f32, kind="ExternalInput")
    bc3_d = nc.dram_tensor("bc3", [1, 1], f32, kind="ExternalInput")
    out_d = nc.dram_tensor("out", [1, 13], f32, kind="ExternalOutput")
    h3dbg_d = nc.dram_tensor("h3dbg", [1, SHARD], f32, kind="ExternalOutput")

    bounce = nc.dram_tensor("bounce", [SHARD, F], f32, kind="Internal")
    u_all = nc.dram_tensor("u_all", [NTOT, F], f32, kind="Internal",
                           addr_space="Shared")
    d6656 = nc.dram_tensor("d6656", [1, SHARD], f32, kind="Internal")
    d208 = nc.dram_tensor("d208", [1, 208], mybir.dt.int16, kind="Internal")

    with tile.TileContext(nc) as tc:
        with (
            tc.tile_pool(name="persist", bufs=1) as pp,
            tc.tile_pool(name="psum_agg", bufs=3, space="PSUM") as ps_agg,
            tc.tile_pool(name="psum_t", bufs=2, space="PSUM") as ps_t,
            tc.tile_pool(name="psum_w", bufs=2, space="PSUM") as ps_w,
        ):
            ident = pp.tile([128, 128], f32)
            make_identity(nc, ident[:])
            gidx = pp.tile([128, CALLS], mybir.dt.int32)
            nc.sync.dma_start(gidx[:], gidx_d[:])
            dinv = pp.tile([128, NROUND], f32)
            nc.sync.dma_start(dinv[:], dinv_d[:])
            Ws, bs = [], []
            for i in range(4):
                w = pp.tile([F, F if i < 3 else 1], f32, name=f"W{i}s")
                nc.sync.dma_start(w[:], W_d[i][:])
                Ws.append(w)
                bb = pp.tile([F if i < 3 else 1, 1], f32, name=f"b{i}s")
                nc.sync.dma_start(bb[:], b_d[i][:])
                bs.append(bb)
            hcatA = pp.tile([128, SHARD], f32)   # h0 (rows 0:64), h1 (64:128)
            hcatB = pp.tile([80, SHARD], f32)    # h2 (0:64), h3 (row 64)

            with tc.tile_pool(name="conv", bufs=1) as cp:
                u_nm = None
                for layer in range(4):
                    # ---- staging: u = dinv * h_prev (node-major) ----
                    u_nm = cp.tile([128, NROUND, F], f32, tag="u_nm",
                                   name=f"u_nm{layer}", bufs=2)
                    if layer == 0:
                        xt = cp.tile([128, NROUND, F], f32, tag="xt", name="xt")
                        nc.sync.dma_start(xt[:], x_nm_d[:])
                        for t in range(NROUND):
                            nc.vector.tensor_tensor(
                                out=u_nm[:, t, :], in0=xt[:, t, :],
                                in1=dinv[:, t:t + 1].to_broadcast([128, F]),
                                op=mybir.AluOpType.mult)
                    else:
                        hprev = (hcatA[0:64, :] if layer == 1 else
                                 hcatA[64:128, :] if layer == 2 else
                                 hcatB[0:64, :])
                        ident64 = (ident[64:128, 64:128] if layer == 2
                                   else ident[0:64, 0:64])
                        for t in range(NROUND):
                            ptile = ps_t.tile([128, 128], f32, tag="tr",
                                              name=f"pt{layer}_{t}")
                            ptile = ptile[:, 0:F]
                            nc.tensor.matmul(
                                ptile[:], hprev[:, t * 128:(t + 1) * 128],
                                ident64, is_transpose=True)
                            nc.vector.tensor_tensor(
                                out=u_nm[:, t, :], in0=ptile[:],
                                in1=dinv[:, t:t + 1].to_broadcast([128, F]),
                                op=mybir.AluOpType.mult)
                    nc.sync.dma_start(
                        bounce.ap().rearrange("(t p) f -> p t f", p=128),
                        u_nm[:])
                    # ---- AllGather the scaled feature table ----
                    nc.gpsimd.collective_compute(
                        "AllGather", mybir.AluOpType.bypass,
                        replica_groups=[list(range(NCORES))],
                        ins=[bounce.ap()], outs=[u_all.ap()])

                    # ---- SpMM: gather + psum accumulate ----
                    s_nm = cp.tile([128, NROUND, F], f32, tag="s_nm",
                                   name=f"s_nm{layer}", bufs=2)
                    for k in range(NROUND):
                        dk = int(D[k])
                        acc = ps_agg.tile([128, F], f32, tag="acc",
                                          name=f"acc{layer}_{k}")
                        nc.tensor.matmul(acc[:], ident[:],
                                         u_nm[:, k, :], start=True,
                                         stop=(dk == 0))
                        for j in range(dk):
                            call = int(callbase[k]) + j
                            gt = cp.tile([128, F], f32, tag="g",
                                         name=f"g{layer}_{k}_{j}", bufs=8)
                            nc.gpsimd.indirect_dma_start(
                                out=gt[:], out_offset=None, in_=u_all[:],
                                in_offset=bass.IndirectOffsetOnAxis(
                                    ap=gidx[:, call:call + 1], axis=0))
                            nc.tensor.matmul(acc[:], ident[:], gt[:],
                                             start=False, stop=(j == dk - 1))
                        nc.vector.tensor_tensor(
                            out=s_nm[:, k, :], in0=acc[:],
                            in1=dinv[:, k:k + 1].to_broadcast([128, F]),
                            op=mybir.AluOpType.mult)

                    # ---- transpose to feature-major ----
                    s_fm = cp.tile([F, SHARD], f32, tag="s_fm",
                                   name=f"s_fm{layer}", bufs=2)
                    for t in range(NROUND):
                        ptile = ps_t.tile([128, 128], f32, tag="tr",
                                          name=f"ptf{layer}_{t}")
                        nc.tensor.matmul(ptile[0:F, :], s_nm[:, t, :], ident[:],
                                         is_transpose=True)
                        nc.scalar.activation(
                            s_fm[:, t * 128:(t + 1) * 128], ptile[0:F, :],
                            AF.Copy)

                    # ---- feature transform + tanh ----
                    fo = F if layer < 3 else 1
                    hout = (hcatA[0:64, :] if layer == 0 else
                            hcatA[64:128, :] if layer == 1 else
                            hcatB[0:64, :] if layer == 2 else
                            hcatB[64:65, :])
                    for t in range(13):
                        pw = ps_w.tile([fo, 512], f32, tag="pw",
                                       name=f"pw{layer}_{t}")
                        nc.tensor.matmul(pw[:], Ws[layer][:],
                                         s_fm[:, t * 512:(t + 1) * 512],
                                         start=True, stop=True)
                        nc.scalar.activation(
                            hout[:, t * 512:(t + 1) * 512], pw[:], AF.Tanh,
                            bias=bs[layer][:])

            # ================= sort-pool + classifier =================
            with tc.tile_pool(name="poolp", bufs=1) as qp:
                h3 = hcatB[64:65, :]
                nc.sync.dma_start(d6656.ap(), h3)
                h3g = qp.tile([13, NPG], f32)
                nc.sync.dma_start(
                    h3g[:],
                    d6656.ap()[:, 0:6500].rearrange("one (g i) -> (one g) i", g=13))

                m8a = qp.tile([13, 8], f32)
                i8a = qp.tile([13, 8], mybir.dt.uint32)
                nc.vector.max(m8a[:], h3g[:])
                nc.vector.max_index(i8a[:], m8a[:], h3g[:])
                h3m = qp.tile([13, NPG], f32)
                nc.vector.match_replace(h3m[:], m8a[:], h3g[:], -2.0)
                m8b = qp.tile([13, 8], f32)
                i8b = qp.tile([13, 8], mybir.dt.uint32)
                nc.vector.max(m8b[:], h3m[:])
                nc.vector.max_index(i8b[:], m8b[:], h3m[:])

                idx2d = qp.tile([13, 16], f32)
                nc.vector.memset(idx2d[:], 0.0)
                nc.vector.tensor_copy(idx2d[:, 0:8], i8a[:])
                nc.vector.tensor_copy(idx2d[:, 8:15], i8b[:, 0:7])
                g500 = qp.tile([13, 1], mybir.dt.int32)
                nc.gpsimd.iota(g500[:], [[0, 1]], base=0, channel_multiplier=NPG)
                g500f = qp.tile([13, 1], f32)
                nc.vector.tensor_copy(g500f[:], g500[:])
                nc.vector.tensor_scalar(
                    out=idx2d[:], in0=idx2d[:], scalar1=g500f[:, 0:1],
                    scalar2=None, op0=mybir.AluOpType.add)
                idx16 = qp.tile([13, 16], mybir.dt.int16)
                nc.vector.tensor_copy(idx16[:], idx2d[:])
                nc.sync.dma_start(
                    d208.ap().rearrange("one (g r) -> (one g) r", g=13),
                    idx16[:])
                idx128 = qp.tile([128, 13], mybir.dt.int16)
                for kk in range(8):
                    nc.sync.dma_start(
                        idx128[kk * 16:(kk + 1) * 16, :],
                        d208.ap().rearrange("one (s p) -> (one p) s", p=16))

                poolA = qp.tile([128, 208], f32)
                nc.gpsimd.ap_gather(poolA[:], hcatA[:], idx128[:],
                                    channels=128, num_elems=SHARD, d=1,
                                    num_idxs=208)
                poolB = qp.tile([80, 208], f32)
                nc.gpsimd.ap_gather(poolB[:], hcatB[:], idx128[0:80, :],
                                    channels=80, num_elems=SHARD, d=1,
                                    num_idxs=208)

                WA = qp.tile([128, K_TOP, 256], f32)
                nc.sync.dma_start(WA[:], WA_d[:])
                WB = qp.tile([80, K_TOP, 256], f32)
                nc.sync.dma_start(WB[:], WB_d[:])
                sc = qp.tile([128, 2], f32)
                nc.sync.dma_start(sc[:], sc_d[:])
                be = qp.tile([128, 2], f32)
                nc.sync.dma_start(be[:], be_d[:])
                Wc1s = qp.tile([128, 2, 128], f32)
                nc.sync.dma_start(Wc1s[:], Wc1_d[:])
                bc1 = qp.tile([128, 1], f32)
                nc.sync.dma_start(bc1[:], bc1_d[:])
                Wc2 = qp.tile([128, F], f32)
                nc.sync.dma_start(Wc2[:], Wc2_d[:])
                bc2 = qp.tile([F, 1], f32)
                nc.sync.dma_start(bc2[:], bc2_d[:])
                Wc3 = qp.tile([F, 1], f32)
                nc.sync.dma_start(Wc3[:], Wc3_d[:])
                bc3 = qp.tile([1, 1], f32)
                nc.sync.dma_start(bc3[:], bc3_d[:])

                # z0 = pooled @ Wc0  (accumulate over 15 rows x 2 K-tiles)
                z1 = []
                for mh in range(2):
                    pz = ps_w.tile([128, 13], f32, tag="pw", name=f"pz{mh}")
                    first = True
                    for r in range(K_TOP):
                        nc.tensor.matmul(
                            pz[:], WA[:, r, mh * 128:(mh + 1) * 128],
                            poolA[:, r:r + 16 * 12 + 1:16],
                            start=first, stop=False)
                        first = False
                        nc.tensor.matmul(
                            pz[:], WB[0:65, r, mh * 128:(mh + 1) * 128],
                            poolB[0:65, r:r + 16 * 12 + 1:16],
                            start=False, stop=(r == K_TOP - 1))
                    zz = qp.tile([128, 13], f32, tag=f"z1_{mh}", name=f"z1_{mh}")
                    nc.scalar.activation(zz[:], pz[:], AF.Relu,
                                         bias=be[:, mh:mh + 1],
                                         scale=sc[:, mh:mh + 1])
                    z1.append(zz)
                pz2 = ps_w.tile([128, 13], f32, tag="pw", name="pz2")
                nc.tensor.matmul(pz2[:], Wc1s[:, 0, :], z1[0][:],
                                 start=True, stop=False)
                nc.tensor.matmul(pz2[:], Wc1s[:, 1, :], z1[1][:],
                                 start=False, stop=True)
                z2 = qp.tile([128, 13], f32)
                nc.scalar.activation(z2[:], pz2[:], AF.Relu, bias=bc1[:])
                pz3 = ps_w.tile([F, 13], f32, tag="pw", name="pz3")
                nc.tensor.matmul(pz3[:], Wc2[:], z2[:], start=True, stop=True)
                z3 = qp.tile([F, 13], f32)
                nc.scalar.activation(z3[:], pz3[:], AF.Relu, bias=bc2[:])
                pz4 = ps_w.tile([1, 13], f32, tag="pw", name="pz4")
                nc.tensor.matmul(pz4[:], Wc3[:], z3[:], start=True, stop=True)
                zf = qp.tile([1, 13], f32)
                nc.vector.tensor_scalar(out=zf[:], in0=pz4[:],
                                        scalar1=bc3[0:1, 0:1], scalar2=None,
                                        op0=mybir.AluOpType.add)
                nc.sync.dma_start(out_d[:], zf[:])
                nc.sync.dma_start(h3dbg_d[:], h3)

    nc.compile()
    return nc


def kernel(**inputs):
    from concourse import bass_utils

    x = np.asarray(inputs["x"], np.float32)
    edge_index = np.asarray(inputs["edge_index"])

    key = ("prog",)
    CALLS, D, callbase, gidxs, x_nms, dinv_nms = _prep(x, edge_index)
    if key in _CACHE and _CACHE[key][0] == CALLS and np.array_equal(_CACHE[key][1], D):
        nc = _CACHE[key][2]
    else:
        nc = _build(CALLS, D, callbase)
        _CACHE[key] = (CALLS, D, nc)

    cw = _pack_classifier(inputs)
    in_maps = []
    for c in range(NCORES):
        m = {
            "x_nm": x_nms[c],
            "gidx": gidxs[c],
            "dinv_nm": dinv_nms[c],
            "WA": cw["WA"], "WB": cw["WB"], "sc": cw["sc"], "be": cw["be"],
            "Wc1s": cw["Wc1s"], "bc1": cw["bc1"], "Wc2": cw["Wc2"],
            "bc2": cw["bc2"], "Wc3": cw["Wc3"], "bc3": cw["bc3"],
        }
        for i in range(4):
            m[f"W{i}"] = np.asarray(inputs[f"W{i}"], np.float32).reshape(
                F, F if i < 3 else 1)
            m[f"b{i}"] = np.asarray(inputs[f"b{i}"], np.float32).reshape(
                F if i < 3 else 1, 1)
        in_maps.append(m)

    trace = os.environ.get("KERNEL_TRACE", "0") == "1"
    kwargs = {}
    if trace:
        import sys, types
        if "antenv.axon_hooks" not in sys.modules:
            sys.path.insert(0, "/root/.axon_site")
            from trn_agent_boot.trn_boot import _ntff_profile_via_ctypes
            mm = types.ModuleType("antenv.axon_hooks")
            mm.get_axon_ntff_profile_hook = (
                lambda: _ntff_profile_via_ctypes("/opt/axon/libaxon_pjrt.so"))
            sys.modules["antenv.axon_hooks"] = mm
        import tempfile
        kwargs = dict(trace=True, tmpdir=tempfile.mkdtemp())

    res = bass_utils.run_bass_kernel_spmd(
        nc, in_maps, core_ids=list(range(NCORES)), **kwargs)

    global LAST_EXEC_NS, LAST_H3
    LAST_EXEC_NS = res.exec_time_ns
    LAST_H3 = [res.results[c]["h3dbg"] for c in range(NCORES)]

    out = np.zeros((G, 1), np.float32)
    for c in range(NCORES):
        ngr = GRAPHS_PER_CORE[c]
        out[GSTART[c]:GSTART[c] + ngr, 0] = res.results[c]["out"][0, :ngr]
    return out


LAST_EXEC_NS = None
LAST_H3 = None



# revision 14
# speedup vs baseline: 1.0817x; 1.0817x over previous
"""DGCNN (4x GCNConv + sort-pool + MLP) on 8 trn2 NeuronCores.

Strategy: graph-parallel sharding (ranks 0-3: 13 graphs, 4-7: 12).
Interleaved slot layout (slot = per-graph degree rank * 13 + graph) so
each 128-dst round holds a narrow degree band across all graphs (fewer
padded slab columns than per-graph blocks) while the sort-pool reload
stays a pure affine DMA. Per layer: u = dinv*h staged node-major (u0
prebuilt on host) -> AllGather full f32 table -> per-slab-column
indirect row gathers into chunked SBUF tiles -> in-place vector-engine
tree reduction per round (+self-loop, dinv scale) -> PE transpose ->
feature transform + tanh per 4 rounds with inline staging of the next
layer's table. Sort-pool via max8/max_index/match_replace, pooled rows
extracted with ap_gather, classifier on PE.
"""
import os
import numpy as np

N = 50000
G = 100
NPG = 500
E = 800000
F = 64
K_TOP = 15
CAT = 193
NCORES = 8
SHARD = 6656
NTOT = SHARD * NCORES
NROUND = SHARD // 128  # 52
BN_EPS = 1e-5
ZROW = 6655            # core-0 pad slot: always-zero table row
CH_MAX = 128           # max gather columns per chunk

GRAPHS_PER_CORE = [13, 13, 13, 13, 12, 12, 12, 12]
GSTART = np.concatenate([[0], np.cumsum(GRAPHS_PER_CORE)])

_CACHE = {}


def _prep(x, edge_index):
    """Host-side sharding/index preprocessing. Pure numpy."""
    src = edge_index[0].astype(np.int64)
    dst = edge_index[1].astype(np.int64)

    deg = np.bincount(dst, minlength=N).astype(np.float32) + np.float32(1.0)
    dinv = deg ** np.float32(-0.5)
    indeg = np.bincount(dst, minlength=N).astype(np.int64)

    node_graph = np.arange(N) // NPG
    node_rank = np.searchsorted(GSTART, node_graph, side="right") - 1  # [N]

    # interleaved slot layout: per-graph degree rank (desc) r, local graph
    # g -> slot = r*13 + g. Rounds then hold narrow degree-quantile bands
    # across all graphs, and the pooling un-permute is a pure affine reload.
    slot_of = np.zeros(N, np.int64)
    for g in range(G):
        lo = g * NPG
        o = np.argsort(-indeg[lo:lo + NPG], kind="stable")
        r = np.empty(NPG, np.int64)
        r[o] = np.arange(NPG)
        g_local = g - GSTART[np.searchsorted(GSTART, g, side="right") - 1]
        slot_of[lo:lo + NPG] = r * 13 + g_local

    pidx = node_rank * SHARD + slot_of  # padded global table row of each node

    # per-core edge lists grouped by dst slot
    e_rank = node_rank[dst]
    counts_all = np.zeros((NCORES, SHARD), np.int64)
    per_core = []
    for c in range(NCORES):
        m = e_rank == c
        s_p = pidx[src[m]].astype(np.int64)
        d_slot = slot_of[dst[m]]
        o = np.argsort(d_slot, kind="stable")
        s_p = s_p[o]
        d_slot = d_slot[o]
        cnt = np.bincount(d_slot, minlength=SHARD)
        counts_all[c] = cnt
        per_core.append((s_p, d_slot, cnt))

    D = counts_all.reshape(NCORES, NROUND, 128).max(axis=(0, 2))  # [NROUND]
    callbase = np.concatenate([[0], np.cumsum(D)])
    CALLS = int(callbase[-1])

    gidxs, u0bs, dinv_nms = [], [], []
    for c in range(NCORES):
        s_p, d_slot, cnt = per_core[c]
        off = np.concatenate([[0], np.cumsum(cnt)])
        gid = np.full((128, CALLS), ZROW, np.int32)
        j_within = np.arange(len(d_slot)) - off[d_slot]
        k = d_slot // 128
        p = d_slot % 128
        call = callbase[k] + j_within
        gid[p, call] = s_p.astype(np.int32)
        gidxs.append(gid)

        nodes = np.arange(NPG * GSTART[c], NPG * GSTART[c + 1])
        u0 = np.zeros((SHARD, F), np.float32)
        u0[slot_of[nodes]] = x[nodes] * dinv[nodes][:, None]
        u0bs.append(u0)

        dv = np.zeros(SHARD, np.float32)
        dv[slot_of[nodes]] = dinv[nodes]
        dinv_nms.append(np.ascontiguousarray(dv.reshape(NROUND, 128).T))

    return CALLS, D, callbase, gidxs, u0bs, dinv_nms


def _chunks(D, callbase):
    """Round-aligned column chunks of at most CH_MAX columns."""
    out = []
    k0 = 0
    while k0 < NROUND:
        k1 = k0
        cols = 0
        while k1 < NROUND and cols + int(D[k1]) <= CH_MAX:
            cols += int(D[k1])
            k1 += 1
        if k1 == k0:  # single round exceeding CH_MAX (cannot happen: D<=128)
            k1 = k0 + 1
            cols = int(D[k0])
        out.append((k0, k1, int(callbase[k0]), int(callbase[k1])))
        k0 = k1
    return out


def _pack_classifier(inp):
    Wc0 = np.asarray(inp["Wc0"], np.float32)           # [2895, 256]
    Wc0r = Wc0.reshape(K_TOP, CAT, 256)
    WA = np.ascontiguousarray(Wc0r[:, 0:128, :].transpose(1, 0, 2))   # [128,15,256]
    WB = np.zeros((80, K_TOP, 256), np.float32)
    WB[0:65] = Wc0r[:, 128:193, :].transpose(1, 0, 2)
    sc_full = np.asarray(inp["gamma"], np.float32) * np.float32(
        1.0 / np.sqrt(1.0 + BN_EPS))
    be_full = (np.asarray(inp["beta"], np.float32)
               + np.asarray(inp["bc0"], np.float32) * sc_full)
    sc = np.ascontiguousarray(sc_full.reshape(2, 128).T)  # [128, 2]
    be = np.ascontiguousarray(be_full.reshape(2, 128).T)
    Wc1 = np.asarray(inp["Wc1"], np.float32)              # [256, 128]
    Wc1s = np.ascontiguousarray(Wc1.reshape(2, 128, 128).transpose(1, 0, 2))
    return {
        "WA": WA, "WB": WB, "sc": sc, "be": be, "Wc1s": Wc1s,
        "bc1": np.asarray(inp["bc1"], np.float32).reshape(128, 1),
        "Wc2": np.asarray(inp["Wc2"], np.float32),
        "bc2": np.asarray(inp["bc2"], np.float32).reshape(64, 1),
        "Wc3": np.asarray(inp["Wc3"], np.float32),
        "bc3": np.asarray(inp["bc3"], np.float32).reshape(1, 1),
    }


def _build(CALLS, D, callbase):
    import concourse.bass as bass
    import concourse.bacc as bacc
    import concourse.mybir as mybir
    from concourse import tile
    from concourse.masks import make_identity

    f32 = mybir.dt.float32
    i32 = mybir.dt.int32
    AF = mybir.ActivationFunctionType
    chunks = _chunks(D, callbase)

    nc = bacc.Bacc("TRN2", target_bir_lowering=False, debug=False,
                   num_devices=NCORES)

    # ---- I/O ----
    u0b_d = nc.dram_tensor("u0b", [SHARD, F], f32, kind="ExternalInput")
    gidx_d = nc.dram_tensor("gidx", [128, CALLS], i32, kind="ExternalInput")
    dinv_d = nc.dram_tensor("dinv_nm", [128, NROUND], f32, kind="ExternalInput")
    W_d = [nc.dram_tensor(f"W{i}", [F, F if i < 3 else 1], f32,
                          kind="ExternalInput") for i in range(4)]
    b_d = [nc.dram_tensor(f"b{i}", [F if i < 3 else 1, 1], f32,
                          kind="ExternalInput") for i in range(4)]
    WA_d = nc.dram_tensor("WA", [128, K_TOP, 256], f32, kind="ExternalInput")
    WB_d = nc.dram_tensor("WB", [80, K_TOP, 256], f32, kind="ExternalInput")
    sc_d = nc.dram_tensor("sc", [128, 2], f32, kind="ExternalInput")
    be_d = nc.dram_tensor("be", [128, 2], f32, kind="ExternalInput")
    Wc1_d = nc.dram_tensor("Wc1s", [128, 2, 128], f32, kind="ExternalInput")
    bc1_d = nc.dram_tensor("bc1", [128, 1], f32, kind="ExternalInput")
    Wc2_d = nc.dram_tensor("Wc2", [128, F], f32, kind="ExternalInput")
    bc2_d = nc.dram_tensor("bc2", [F, 1], f32, kind="ExternalInput")
    Wc3_d = nc.dram_tensor("Wc3", [F, 1], f32, kind="ExternalInput")
    bc3_d = nc.dram_tensor("bc3", [1, 1], f32, kind="ExternalInput")
    out_d = nc.dram_tensor("out", [1, 13], f32, kind="ExternalOutput")

    bounce = nc.dram_tensor("bounce", [SHARD, F], f32, kind="Internal")
    u_all = nc.dram_tensor("u_all", [NTOT, F], f32, kind="Internal",
                           addr_space="Shared")
    d6656 = nc.dram_tensor("d6656", [1, SHARD], f32, kind="Internal")
    d208 = nc.dram_tensor("d208", [1, 208], mybir.dt.int16, kind="Internal")

    with tile.TileContext(nc) as tc:
        with (
            tc.tile_pool(name="persist", bufs=1) as pp,
            tc.tile_pool(name="psum_t", bufs=4, space="PSUM") as ps_t,
            tc.tile_pool(name="psum_w", bufs=2, space="PSUM") as ps_w,
        ):
            ident = pp.tile([128, 128], f32)
            make_identity(nc, ident[:])
            gidx = pp.tile([128, CALLS], i32)
            nc.sync.dma_start(gidx[:], gidx_d[:])
            dinv = pp.tile([128, NROUND], f32)
            nc.sync.dma_start(dinv[:], dinv_d[:])
            Ws, bs = [], []
            for i in range(4):
                w = pp.tile([F, F if i < 3 else 1], f32, name=f"W{i}s")
                nc.sync.dma_start(w[:], W_d[i][:])
                Ws.append(w)
                bb = pp.tile([F if i < 3 else 1, 1], f32, name=f"b{i}s")
                nc.sync.dma_start(bb[:], b_d[i][:])
                bs.append(bb)
            hcatA = pp.tile([128, SHARD], f32)   # h0 (rows 0:64), h1 (64:128)
            hcatB = pp.tile([80, SHARD], f32)    # h2 (0:64), h3 (row 64)

            with tc.tile_pool(name="conv", bufs=1) as cp:
                u_nm = cp.tile([128, NROUND, F], f32, tag="u_nm",
                               name="u_nm0", bufs=2)
                nc.sync.dma_start(
                    u_nm[:], u0b_d.ap().rearrange("(t p) f -> p t f", p=128))
                nc.sync.dma_start(bounce.ap(), u0b_d.ap())
                for layer in range(4):
                    # ---- AllGather the scaled feature table ----
                    nc.gpsimd.collective_compute(
                        "AllGather", mybir.AluOpType.bypass,
                        replica_groups=[list(range(NCORES))],
                        ins=[bounce.ap()], outs=[u_all.ap()])

                    fo = F if layer < 3 else 1
                    hout = (hcatA[0:64, :] if layer == 0 else
                            hcatA[64:128, :] if layer == 1 else
                            hcatB[0:64, :] if layer == 2 else
                            hcatB[64:65, :])
                    if layer < 3:
                        u_next = cp.tile([128, NROUND, F], f32, tag="u_nm",
                                         name=f"u_nm{layer + 1}", bufs=2)
                    s_fm = cp.tile([F, SHARD], f32, tag="s_fm",
                                   name=f"s_fm{layer}", bufs=1)

                    for (k0, k1, c0, c1) in chunks:
                        cols = c1 - c0
                        gt = cp.tile([128, CH_MAX, F], f32, tag="gt",
                                     name=f"gt{layer}_{k0}", bufs=2)
                        for c in range(c0, c1):
                            nc.gpsimd.indirect_dma_start(
                                out=gt[:, c - c0, :], out_offset=None,
                                in_=u_all[:],
                                in_offset=bass.IndirectOffsetOnAxis(
                                    ap=gidx[:, c:c + 1], axis=0))
                        for k in range(k0, k1):
                            dk = int(D[k])
                            q0 = int(callbase[k]) - c0
                            # in-place tree reduction of dk slab columns
                            d_cur = dk
                            while d_cur > 1:
                                h = d_cur // 2
                                nc.vector.tensor_tensor(
                                    out=gt[:, q0:q0 + h, :],
                                    in0=gt[:, q0:q0 + h, :],
                                    in1=gt[:, q0 + d_cur - h:q0 + d_cur, :],
                                    op=mybir.AluOpType.add)
                                d_cur -= h
                            s_nm = cp.tile([128, F], f32, tag="s_nm",
                                           name=f"s_nm{layer}_{k}", bufs=4)
                            if dk > 0:
                                nc.vector.tensor_tensor(
                                    out=s_nm[:], in0=gt[:, q0, :],
                                    in1=u_nm[:, k, :],
                                    op=mybir.AluOpType.add)
                            else:
                                nc.vector.tensor_copy(s_nm[:], u_nm[:, k, :])
                            nc.vector.tensor_tensor(
                                out=s_nm[:], in0=s_nm[:],
                                in1=dinv[:, k:k + 1].to_broadcast([128, F]),
                                op=mybir.AluOpType.mult)
                            # transpose to feature-major
                            ptile = ps_t.tile([F, 128], f32, tag="trf",
                                              name=f"ptf{layer}_{k}", bufs=2)
                            nc.tensor.matmul(ptile[:], s_nm[:], ident[:],
                                             is_transpose=True)
                            nc.scalar.activation(
                                s_fm[:, k * 128:(k + 1) * 128], ptile[:],
                                AF.Copy)
                            # transform + tanh + inline staging per 4 rounds
                            if k % 4 == 3 or k == NROUND - 1:
                                t0 = (k // 4) * 4
                                nc0 = t0 * 128
                                nc1 = (k + 1) * 128
                                pw = ps_w.tile([fo, 512], f32, tag="pw",
                                               name=f"pw{layer}_{t0}")
                                nc.tensor.matmul(pw[:, 0:nc1 - nc0],
                                                 Ws[layer][:],
                                                 s_fm[:, nc0:nc1],
                                                 start=True, stop=True)
                                nc.scalar.activation(
                                    hout[:, nc0:nc1], pw[:, 0:nc1 - nc0],
                                    AF.Tanh, bias=bs[layer][:])
                                if layer < 3:
                                    for t in range(t0, k + 1):
                                        pt2 = ps_t.tile(
                                            [128, F], f32, tag="trs",
                                            name=f"pts{layer}_{t}", bufs=2)
                                        ident64 = (ident[64:128, 64:128]
                                                   if layer == 1
                                                   else ident[0:F, 0:F])
                                        nc.tensor.matmul(
                                            pt2[:],
                                            hout[:, t * 128:(t + 1) * 128],
                                            ident64,
                                            is_transpose=True)
                                        nc.vector.tensor_tensor(
                                            out=u_next[:, t, :], in0=pt2[:],
                                            in1=dinv[:, t:t + 1]
                                            .to_broadcast([128, F]),
                                            op=mybir.AluOpType.mult)
                    if layer < 3:
                        nc.sync.dma_start(
                            bounce.ap().rearrange("(t p) f -> p t f", p=128),
                            u_next[:])
                        u_nm = u_next

            # ================= sort-pool + classifier =================
            with tc.tile_pool(name="poolp", bufs=1) as qp:
                h3 = hcatB[64:65, :]
                nc.sync.dma_start(d6656.ap(), h3)
                h3gt = qp.tile([13, NPG], f32)
                nc.sync.dma_start(
                    h3gt[:],
                    d6656.ap()[:, 0:6500].rearrange(
                        "one (i g) -> (one g) i", g=13))
                h3g = h3gt[:]

                m8a = qp.tile([13, 8], f32)
                i8a = qp.tile([13, 8], mybir.dt.uint32)
                nc.vector.max(m8a[:], h3g)
                nc.vector.max_index(i8a[:], m8a[:], h3g)
                h3m = qp.tile([13, NPG], f32)
                nc.vector.match_replace(h3m[:], m8a[:], h3g, -2.0)
                m8b = qp.tile([13, 8], f32)
                i8b = qp.tile([13, 8], mybir.dt.uint32)
                nc.vector.max(m8b[:], h3m[:])
                nc.vector.max_index(i8b[:], m8b[:], h3m[:])

                idx2d = qp.tile([13, 16], f32)
                nc.vector.memset(idx2d[:], 0.0)
                nc.vector.tensor_copy(idx2d[:, 0:8], i8a[:])
                nc.vector.tensor_copy(idx2d[:, 8:15], i8b[:, 0:7])
                gof = qp.tile([13, 1], mybir.dt.int32)
                nc.gpsimd.iota(gof[:], [[0, 1]], base=0, channel_multiplier=1)
                goff = qp.tile([13, 1], f32)
                nc.vector.tensor_copy(goff[:], gof[:])
                # absolute slot = pos*13 + g
                nc.vector.tensor_scalar(
                    out=idx2d[:], in0=idx2d[:], scalar1=13.0,
                    scalar2=None, op0=mybir.AluOpType.mult)
                nc.vector.tensor_scalar(
                    out=idx2d[:], in0=idx2d[:], scalar1=goff[:, 0:1],
                    scalar2=None, op0=mybir.AluOpType.add)
                idx16 = qp.tile([13, 16], mybir.dt.int16)
                nc.vector.tensor_copy(idx16[:], idx2d[:])
                nc.sync.dma_start(
                    d208.ap().rearrange("one (g r) -> (one g) r", g=13),
                    idx16[:])
                idx128 = qp.tile([128, 13], mybir.dt.int16)
                for kk in range(8):
                    nc.sync.dma_start(
                        idx128[kk * 16:(kk + 1) * 16, :],
                        d208.ap().rearrange("one (s p) -> (one p) s", p=16))

                poolA = qp.tile([128, 208], f32)
                nc.gpsimd.ap_gather(poolA[:], hcatA[:], idx128[:],
                                    channels=128, num_elems=SHARD, d=1,
                                    num_idxs=208)
                poolB = qp.tile([80, 208], f32)
                nc.gpsimd.ap_gather(poolB[:], hcatB[:], idx128[0:80, :],
                                    channels=80, num_elems=SHARD, d=1,
                                    num_idxs=208)

                WA = qp.tile([128, K_TOP, 256], f32)
                nc.sync.dma_start(WA[:], WA_d[:])
                WB = qp.tile([80, K_TOP, 256], f32)
                nc.sync.dma_start(WB[:], WB_d[:])
                sc = qp.tile([128, 2], f32)
                nc.sync.dma_start(sc[:], sc_d[:])
                be = qp.tile([128, 2], f32)
                nc.sync.dma_start(be[:], be_d[:])
                Wc1s = qp.tile([128, 2, 128], f32)
                nc.sync.dma_start(Wc1s[:], Wc1_d[:])
                bc1 = qp.tile([128, 1], f32)
                nc.sync.dma_start(bc1[:], bc1_d[:])
                Wc2 = qp.tile([128, F], f32)
                nc.sync.dma_start(Wc2[:], Wc2_d[:])
                bc2 = qp.tile([F, 1], f32)
                nc.sync.dma_start(bc2[:], bc2_d[:])
                Wc3 = qp.tile([F, 1], f32)
                nc.sync.dma_start(Wc3[:], Wc3_d[:])
                bc3 = qp.tile([1, 1], f32)
                nc.sync.dma_start(bc3[:], bc3_d[:])

                z1 = []
                for mh in range(2):
                    pz = ps_w.tile([128, 13], f32, tag="pw", name=f"pz{mh}")
                    first = True
                    for r in range(K_TOP):
                        nc.tensor.matmul(
                            pz[:], WA[:, r, mh * 128:(mh + 1) * 128],
                            poolA[:, r:r + 16 * 12 + 1:16],
                            start=first, stop=False)
                        first = False
                        nc.tensor.matmul(
                            pz[:], WB[0:65, r, mh * 128:(mh + 1) * 128],
                            poolB[0:65, r:r + 16 * 12 + 1:16],
                            start=False, stop=(r == K_TOP - 1))
                    zz = qp.tile([128, 13], f32, tag=f"z1_{mh}", name=f"z1_{mh}")
                    nc.scalar.activation(zz[:], pz[:], AF.Relu,
                                         bias=be[:, mh:mh + 1],
                                         scale=sc[:, mh:mh + 1])
                    z1.append(zz)
                pz2 = ps_w.tile([128, 13], f32, tag="pw", name="pz2")
                nc.tensor.matmul(pz2[:], Wc1s[:, 0, :], z1[0][:],
                                 start=True, stop=False)
                nc.tensor.matmul(pz2[:], Wc1s[:, 1, :], z1[1][:],
                                 start=False, stop=True)
                z2 = qp.tile([128, 13], f32)
                nc.scalar.activation(z2[:], pz2[:], AF.Relu, bias=bc1[:])
                pz3 = ps_w.tile([F, 13], f32, tag="pw", name="pz3")
                nc.tensor.matmul(pz3[:], Wc2[:], z2[:], start=True, stop=True)
                z3 = qp.tile([F, 13], f32)
                nc.scalar.activation(z3[:], pz3[:], AF.Relu, bias=bc2[:])
                pz4 = ps_w.tile([1, 13], f32, tag="pw", name="pz4")
                nc.tensor.matmul(pz4[:], Wc3[:], z3[:], start=True, stop=True)
                zf = qp.tile([1, 13], f32)
                nc.vector.tensor_scalar(out=zf[:], in0=pz4[:],
                                        scalar1=bc3[0:1, 0:1], scalar2=None,
                                        op0=mybir.AluOpType.add)
                nc.sync.dma_start(out_d[:], zf[:])

    nc.compile()
    return nc


def kernel(**inputs):
    from concourse import bass_utils

    x = np.asarray(inputs["x"], np.float32)
    edge_index = np.asarray(inputs["edge_index"])

    key = ("prog",)
    CALLS, D, callbase, gidxs, u0bs, dinv_nms = _prep(x, edge_index)
    if key in _CACHE and _CACHE[key][0] == CALLS and np.array_equal(_CACHE[key][1], D):
        nc = _CACHE[key][2]
    else:
        nc = _build(CALLS, D, callbase)
        _CACHE[key] = (CALLS, D, nc)

    cw = _pack_classifier(inputs)
    in_maps = []
    for c in range(NCORES):
        m = {
            "u0b": u0bs[c],
            "gidx": gidxs[c],
            "dinv_nm": dinv_nms[c],
            "WA": cw["WA"], "WB": cw["WB"], "sc": cw["sc"], "be": cw["be"],
            "Wc1s": cw["Wc1s"], "bc1": cw["bc1"], "Wc2": cw["Wc2"],
            "bc2": cw["bc2"], "Wc3": cw["Wc3"], "bc3": cw["bc3"],
        }
        for i in range(4):
            m[f"W{i}"] = np.asarray(inputs[f"W{i}"], np.float32).reshape(
                F, F if i < 3 else 1)
            m[f"b{i}"] = np.asarray(inputs[f"b{i}"], np.float32).reshape(
                F if i < 3 else 1, 1)
        in_maps.append(m)

    trace = os.environ.get("KERNEL_TRACE", "0") == "1"
    kwargs = {}
    if trace:
        import sys, types
        if "antenv.axon_hooks" not in sys.modules:
            sys.path.insert(0, "/root/.axon_site")
            from trn_agent_boot.trn_boot import _ntff_profile_via_ctypes
            mm = types.ModuleType("antenv.axon_hooks")
            mm.get_axon_ntff_profile_hook = (
                lambda: _ntff_profile_via_ctypes("/opt/axon/libaxon_pjrt.so"))
            sys.modules["antenv.axon_hooks"] = mm
        import tempfile
        kwargs = dict(trace=True, tmpdir=tempfile.mkdtemp())

    res = bass_utils.run_bass_kernel_spmd(
        nc, in_maps, core_ids=list(range(NCORES)), **kwargs)

    global LAST_EXEC_NS
    LAST_EXEC_NS = res.exec_time_ns

    out = np.zeros((G, 1), np.float32)
    for c in range(NCORES):
        ngr = GRAPHS_PER_CORE[c]
        out[GSTART[c]:GSTART[c] + ngr, 0] = res.results[c]["out"][0, :ngr]
    return out


LAST_EXEC_NS = None


# revision 15
# speedup vs baseline: 1.2700x; 1.1741x over previous
"""DGCNN (4x GCNConv + sort-pool + MLP) on 8 trn2 NeuronCores.

Strategy: graph-parallel sharding (ranks 0-3: 13 graphs, 4-7: 12).
Interleaved slot layout (slot = per-graph degree rank * 13 + graph) so
each 128-dst round holds a narrow degree band across all graphs (fewer
padded slab columns than per-graph blocks) while the sort-pool reload
stays a pure affine DMA. Per layer: u = dinv*h staged node-major (u0
prebuilt on host) -> AllGather full f32 table -> per-slab-column
indirect row gathers into chunked SBUF tiles -> in-place vector-engine
tree reduction per round (+self-loop, dinv scale) -> PE transpose ->
feature transform + tanh per 4 rounds with inline staging of the next
layer's table. Sort-pool via max8/max_index/match_replace, pooled rows
extracted with ap_gather, classifier on PE.
"""
import os
import numpy as np

N = 50000
G = 100
NPG = 500
E = 800000
F = 64
K_TOP = 15
CAT = 193
NCORES = 8
SHARD = 6656
NTOT = SHARD * NCORES
NROUND = SHARD // 128  # 52
BN_EPS = 1e-5
ZROW = 29951           # core-0 pad slot 6655 in split-table rows
CH_MAX = 112           # max gather columns per chunk

GRAPHS_PER_CORE = [13, 13, 13, 13, 12, 12, 12, 12]
GSTART = np.concatenate([[0], np.cumsum(GRAPHS_PER_CORE)])

_CACHE = {}


def _prep(x, edge_index):
    """Host-side sharding/index preprocessing. Pure numpy."""
    src = edge_index[0].astype(np.int64)
    dst = edge_index[1].astype(np.int64)

    deg = np.bincount(dst, minlength=N).astype(np.float32) + np.float32(1.0)
    dinv = deg ** np.float32(-0.5)
    indeg = np.bincount(dst, minlength=N).astype(np.int64)

    node_graph = np.arange(N) // NPG
    node_rank = np.searchsorted(GSTART, node_graph, side="right") - 1  # [N]

    # interleaved slot layout: per-graph degree rank (desc) r, local graph
    # g -> slot = r*13 + g. Rounds then hold narrow degree-quantile bands
    # across all graphs, and the pooling un-permute is a pure affine reload.
    slot_of = np.zeros(N, np.int64)
    for g in range(G):
        lo = g * NPG
        o = np.argsort(-indeg[lo:lo + NPG], kind="stable")
        r = np.empty(NPG, np.int64)
        r[o] = np.arange(NPG)
        g_local = g - GSTART[np.searchsorted(GSTART, g, side="right") - 1]
        slot_of[lo:lo + NPG] = r * 13 + g_local

    # split-table rows: half A = slots 0:3328 of every rank (gathered by
    # collective A), half B = slots 3328:. Row = half*26624 + rank*3328 +
    # (slot - half*3328).
    half = (slot_of >= 3328).astype(np.int64)
    pidx = half * 26624 + node_rank * 3328 + (slot_of - half * 3328)

    # per-core edge lists grouped by dst slot
    e_rank = node_rank[dst]
    counts_all = np.zeros((NCORES, SHARD), np.int64)
    per_core = []
    for c in range(NCORES):
        m = e_rank == c
        s_p = pidx[src[m]].astype(np.int64)
        d_slot = slot_of[dst[m]]
        o = np.argsort(d_slot, kind="stable")
        s_p = s_p[o]
        d_slot = d_slot[o]
        cnt = np.bincount(d_slot, minlength=SHARD)
        counts_all[c] = cnt
        per_core.append((s_p, d_slot, cnt))

    D = counts_all.reshape(NCORES, NROUND, 128).max(axis=(0, 2))  # [NROUND]
    callbase = np.concatenate([[0], np.cumsum(D)])
    CALLS = int(callbase[-1])

    gidxs, u0bs, dinv_nms = [], [], []
    for c in range(NCORES):
        s_p, d_slot, cnt = per_core[c]
        off = np.concatenate([[0], np.cumsum(cnt)])
        gid = np.full((128, CALLS), ZROW, np.int32)
        j_within = np.arange(len(d_slot)) - off[d_slot]
        k = d_slot // 128
        p = d_slot % 128
        call = callbase[k] + j_within
        gid[p, call] = s_p.astype(np.int32)
        gidxs.append(gid)

        nodes = np.arange(NPG * GSTART[c], NPG * GSTART[c + 1])
        u0 = np.zeros((SHARD, F), np.float32)
        u0[slot_of[nodes]] = x[nodes] * dinv[nodes][:, None]
        u0bs.append(u0)

        dv = np.zeros(SHARD, np.float32)
        dv[slot_of[nodes]] = dinv[nodes]
        dinv_nms.append(np.ascontiguousarray(dv.reshape(NROUND, 128).T))

    return CALLS, D, callbase, gidxs, u0bs, dinv_nms


def _chunks(D, callbase):
    """Round-aligned column chunks of at most CH_MAX columns."""
    out = []
    k0 = 0
    while k0 < NROUND:
        k1 = k0
        cols = 0
        while k1 < NROUND and cols + int(D[k1]) <= CH_MAX:
            cols += int(D[k1])
            k1 += 1
        if k1 == k0:  # single round exceeding CH_MAX (cannot happen: D<=128)
            k1 = k0 + 1
            cols = int(D[k0])
        out.append((k0, k1, int(callbase[k0]), int(callbase[k1])))
        k0 = k1
    return out


def _pack_classifier(inp):
    Wc0 = np.asarray(inp["Wc0"], np.float32)           # [2895, 256]
    Wc0r = Wc0.reshape(K_TOP, CAT, 256)
    WA = np.ascontiguousarray(Wc0r[:, 0:128, :].transpose(1, 0, 2))   # [128,15,256]
    WB = np.zeros((80, K_TOP, 256), np.float32)
    WB[0:65] = Wc0r[:, 128:193, :].transpose(1, 0, 2)
    sc_full = np.asarray(inp["gamma"], np.float32) * np.float32(
        1.0 / np.sqrt(1.0 + BN_EPS))
    be_full = (np.asarray(inp["beta"], np.float32)
               + np.asarray(inp["bc0"], np.float32) * sc_full)
    sc = np.ascontiguousarray(sc_full.reshape(2, 128).T)  # [128, 2]
    be = np.ascontiguousarray(be_full.reshape(2, 128).T)
    Wc1 = np.asarray(inp["Wc1"], np.float32)              # [256, 128]
    Wc1s = np.ascontiguousarray(Wc1.reshape(2, 128, 128).transpose(1, 0, 2))
    return {
        "WA": WA, "WB": WB, "sc": sc, "be": be, "Wc1s": Wc1s,
        "bc1": np.asarray(inp["bc1"], np.float32).reshape(128, 1),
        "Wc2": np.asarray(inp["Wc2"], np.float32),
        "bc2": np.asarray(inp["bc2"], np.float32).reshape(64, 1),
        "Wc3": np.asarray(inp["Wc3"], np.float32),
        "bc3": np.asarray(inp["bc3"], np.float32).reshape(1, 1),
    }


def _build(CALLS, D, callbase):
    import concourse.bass as bass
    import concourse.bacc as bacc
    import concourse.mybir as mybir
    from concourse import tile
    from concourse.masks import make_identity

    f32 = mybir.dt.float32
    i32 = mybir.dt.int32
    AF = mybir.ActivationFunctionType
    chunks = _chunks(D, callbase)

    nc = bacc.Bacc("TRN2", target_bir_lowering=False, debug=False,
                   num_devices=NCORES)

    # ---- I/O ----
    u0b_d = nc.dram_tensor("u0b", [SHARD, F], f32, kind="ExternalInput")
    gidx_d = nc.dram_tensor("gidx", [128, CALLS], i32, kind="ExternalInput")
    dinv_d = nc.dram_tensor("dinv_nm", [128, NROUND], f32, kind="ExternalInput")
    W_d = [nc.dram_tensor(f"W{i}", [F, F if i < 3 else 1], f32,
                          kind="ExternalInput") for i in range(4)]
    b_d = [nc.dram_tensor(f"b{i}", [F if i < 3 else 1, 1], f32,
                          kind="ExternalInput") for i in range(4)]
    WA_d = nc.dram_tensor("WA", [128, K_TOP, 256], f32, kind="ExternalInput")
    WB_d = nc.dram_tensor("WB", [80, K_TOP, 256], f32, kind="ExternalInput")
    sc_d = nc.dram_tensor("sc", [128, 2], f32, kind="ExternalInput")
    be_d = nc.dram_tensor("be", [128, 2], f32, kind="ExternalInput")
    Wc1_d = nc.dram_tensor("Wc1s", [128, 2, 128], f32, kind="ExternalInput")
    bc1_d = nc.dram_tensor("bc1", [128, 1], f32, kind="ExternalInput")
    Wc2_d = nc.dram_tensor("Wc2", [128, F], f32, kind="ExternalInput")
    bc2_d = nc.dram_tensor("bc2", [F, 1], f32, kind="ExternalInput")
    Wc3_d = nc.dram_tensor("Wc3", [F, 1], f32, kind="ExternalInput")
    bc3_d = nc.dram_tensor("bc3", [1, 1], f32, kind="ExternalInput")
    out_d = nc.dram_tensor("out", [1, 13], f32, kind="ExternalOutput")

    bounceA = nc.dram_tensor("bounceA", [3328, F], f32, kind="Internal")
    bounceB = nc.dram_tensor("bounceB", [3328, F], f32, kind="Internal")
    u_alls = [nc.dram_tensor(f"u_all{i}", [NTOT, F], f32, kind="Internal",
                             addr_space="Shared") for i in range(2)]
    d6656 = nc.dram_tensor("d6656", [1, SHARD], f32, kind="Internal")
    d208 = nc.dram_tensor("d208", [1, 208], mybir.dt.int16, kind="Internal")

    with tile.TileContext(nc) as tc:
        with (
            tc.tile_pool(name="persist", bufs=1) as pp,
            tc.tile_pool(name="psum_t", bufs=4, space="PSUM") as ps_t,
            tc.tile_pool(name="psum_w", bufs=2, space="PSUM") as ps_w,
        ):
            ident = pp.tile([128, 128], f32)
            make_identity(nc, ident[:])
            gidx = pp.tile([128, CALLS], i32)
            nc.sync.dma_start(gidx[:], gidx_d[:])
            dinv = pp.tile([128, NROUND], f32)
            nc.sync.dma_start(dinv[:], dinv_d[:])
            Ws, bs = [], []
            for i in range(4):
                w = pp.tile([F, F if i < 3 else 1], f32, name=f"W{i}s")
                nc.sync.dma_start(w[:], W_d[i][:])
                Ws.append(w)
                bb = pp.tile([F if i < 3 else 1, 1], f32, name=f"b{i}s")
                nc.sync.dma_start(bb[:], b_d[i][:])
                bs.append(bb)
            hcatA = pp.tile([128, SHARD], f32)   # h0 (rows 0:64), h1 (64:128)
            hcatB = pp.tile([80, SHARD], f32)    # h2 (0:64), h3 (row 64)

            with tc.tile_pool(name="conv", bufs=1) as cp:
                u_nm = cp.tile([128, NROUND, F], f32, tag="u_nm",
                               name="u_nm0", bufs=2)
                nc.sync.dma_start(
                    u_nm[:], u0b_d.ap().rearrange("(t p) f -> p t f", p=128))
                nc.sync.dma_start(bounceA.ap(), u0b_d.ap()[0:3328, :])
                nc.sync.dma_start(bounceB.ap(), u0b_d.ap()[3328:SHARD, :])

                def coll_half(dst, which):
                    src = bounceA if which == 0 else bounceB
                    lo = which * 26624
                    nc.gpsimd.collective_compute(
                        "AllGather", mybir.AluOpType.bypass,
                        replica_groups=[list(range(NCORES))],
                        ins=[src.ap()], outs=[dst.ap()[lo:lo + 26624, :]])

                coll_half(u_alls[0], 0)
                coll_half(u_alls[0], 1)
                for layer in range(4):
                    u_all = u_alls[layer % 2]

                    fo = F if layer < 3 else 1
                    hout = (hcatA[0:64, :] if layer == 0 else
                            hcatA[64:128, :] if layer == 1 else
                            hcatB[0:64, :] if layer == 2 else
                            hcatB[64:65, :])
                    if layer < 3:
                        u_next = cp.tile([128, NROUND, F], f32, tag="u_nm",
                                         name=f"u_nm{layer + 1}", bufs=2)
                    s_fm = cp.tile([F, SHARD], f32, tag="s_fm",
                                   name=f"s_fm{layer}", bufs=1)

                    collA_issued = False
                    for (k0, k1, c0, c1) in chunks:
                        cols = c1 - c0
                        gt = cp.tile([128, CH_MAX, F], f32, tag="gt",
                                     name=f"gt{layer}_{k0}", bufs=2)
                        for c in range(c0, c1):
                            nc.gpsimd.indirect_dma_start(
                                out=gt[:, c - c0, :], out_offset=None,
                                in_=u_all[:],
                                in_offset=bass.IndirectOffsetOnAxis(
                                    ap=gidx[:, c:c + 1], axis=0))
                        for k in range(k0, k1):
                            dk = int(D[k])
                            q0 = int(callbase[k]) - c0
                            # in-place tree reduction of dk slab columns
                            d_cur = dk
                            while d_cur > 1:
                                h = d_cur // 2
                                nc.vector.tensor_tensor(
                                    out=gt[:, q0:q0 + h, :],
                                    in0=gt[:, q0:q0 + h, :],
                                    in1=gt[:, q0 + d_cur - h:q0 + d_cur, :],
                                    op=mybir.AluOpType.add)
                                d_cur -= h
                            s_nm = cp.tile([128, F], f32, tag="s_nm",
                                           name=f"s_nm{layer}_{k}", bufs=4)
                            if dk > 0:
                                nc.vector.tensor_tensor(
                                    out=s_nm[:], in0=gt[:, q0, :],
                                    in1=u_nm[:, k, :],
                                    op=mybir.AluOpType.add)
                            else:
                                nc.vector.tensor_copy(s_nm[:], u_nm[:, k, :])
                            nc.vector.tensor_tensor(
                                out=s_nm[:], in0=s_nm[:],
                                in1=dinv[:, k:k + 1].to_broadcast([128, F]),
                                op=mybir.AluOpType.mult)
                            # transpose to feature-major
                            ptile = ps_t.tile([F, 128], f32, tag="trf",
                                              name=f"ptf{layer}_{k}", bufs=2)
                            nc.tensor.matmul(ptile[:], s_nm[:], ident[:],
                                             is_transpose=True)
                            nc.scalar.activation(
                                s_fm[:, k * 128:(k + 1) * 128], ptile[:],
                                AF.Copy)
                            # transform + tanh + inline staging per 4 rounds
                            if k % 4 == 3 or k == NROUND - 1:
                                t0 = (k // 4) * 4
                                nc0 = t0 * 128
                                nc1 = (k + 1) * 128
                                pw = ps_w.tile([fo, 512], f32, tag="pw",
                                               name=f"pw{layer}_{t0}")
                                nc.tensor.matmul(pw[:, 0:nc1 - nc0],
                                                 Ws[layer][:],
                                                 s_fm[:, nc0:nc1],
                                                 start=True, stop=True)
                                nc.scalar.activation(
                                    hout[:, nc0:nc1], pw[:, 0:nc1 - nc0],
                                    AF.Tanh, bias=bs[layer][:])
                                if layer < 3:
                                    for t in range(t0, k + 1):
                                        pt2 = ps_t.tile(
                                            [128, F], f32, tag="trs",
                                            name=f"pts{layer}_{t}", bufs=2)
                                        ident64 = (ident[64:128, 64:128]
                                                   if layer == 1
                                                   else ident[0:F, 0:F])
                                        nc.tensor.matmul(
                                            pt2[:],
                                            hout[:, t * 128:(t + 1) * 128],
                                            ident64,
                                            is_transpose=True)
                                        nc.vector.tensor_tensor(
                                            out=u_next[:, t, :], in0=pt2[:],
                                            in1=dinv[:, t:t + 1]
                                            .to_broadcast([128, F]),
                                            op=mybir.AluOpType.mult)
                        if layer < 3 and not collA_issued and k1 >= 28:
                            # rounds 0-27 staged: ship half A of next table
                            # while the rest of this layer still gathers.
                            nc.sync.dma_start(
                                bounceA.ap().rearrange(
                                    "(t p) f -> p t f", p=128),
                                u_next[:, 0:26, :])
                            coll_half(u_alls[(layer + 1) % 2], 0)
                            collA_issued = True
                    if layer < 3:
                        nc.sync.dma_start(
                            bounceB.ap().rearrange("(t p) f -> p t f", p=128),
                            u_next[:, 26:NROUND, :])
                        coll_half(u_alls[(layer + 1) % 2], 1)
                        u_nm = u_next

            # ================= sort-pool + classifier =================
            with tc.tile_pool(name="poolp", bufs=1) as qp:
                h3 = hcatB[64:65, :]
                nc.sync.dma_start(d6656.ap(), h3)
                h3gt = qp.tile([13, NPG], f32)
                nc.sync.dma_start(
                    h3gt[:],
                    d6656.ap()[:, 0:6500].rearrange(
                        "one (i g) -> (one g) i", g=13))
                h3g = h3gt[:]

                m8a = qp.tile([13, 8], f32)
                i8a = qp.tile([13, 8], mybir.dt.uint32)
                nc.vector.max(m8a[:], h3g)
                nc.vector.max_index(i8a[:], m8a[:], h3g)
                h3m = qp.tile([13, NPG], f32)
                nc.vector.match_replace(h3m[:], m8a[:], h3g, -2.0)
                m8b = qp.tile([13, 8], f32)
                i8b = qp.tile([13, 8], mybir.dt.uint32)
                nc.vector.max(m8b[:], h3m[:])
                nc.vector.max_index(i8b[:], m8b[:], h3m[:])

                idx2d = qp.tile([13, 16], f32)
                nc.vector.memset(idx2d[:], 0.0)
                nc.vector.tensor_copy(idx2d[:, 0:8], i8a[:])
                nc.vector.tensor_copy(idx2d[:, 8:15], i8b[:, 0:7])
                gof = qp.tile([13, 1], mybir.dt.int32)
                nc.gpsimd.iota(gof[:], [[0, 1]], base=0, channel_multiplier=1)
                goff = qp.tile([13, 1], f32)
                nc.vector.tensor_copy(goff[:], gof[:])
                # absolute slot = pos*13 + g
                nc.vector.tensor_scalar(
                    out=idx2d[:], in0=idx2d[:], scalar1=13.0,
                    scalar2=None, op0=mybir.AluOpType.mult)
                nc.vector.tensor_scalar(
                    out=idx2d[:], in0=idx2d[:], scalar1=goff[:, 0:1],
                    scalar2=None, op0=mybir.AluOpType.add)
                idx16 = qp.tile([13, 16], mybir.dt.int16)
                nc.vector.tensor_copy(idx16[:], idx2d[:])
                nc.sync.dma_start(
                    d208.ap().rearrange("one (g r) -> (one g) r", g=13),
                    idx16[:])
                idx128 = qp.tile([128, 13], mybir.dt.int16)
                for kk in range(8):
                    nc.sync.dma_start(
                        idx128[kk * 16:(kk + 1) * 16, :],
                        d208.ap().rearrange("one (s p) -> (one p) s", p=16))

                poolA = qp.tile([128, 208], f32)
                nc.gpsimd.ap_gather(poolA[:], hcatA[:], idx128[:],
                                    channels=128, num_elems=SHARD, d=1,
                                    num_idxs=208)
                poolB = qp.tile([80, 208], f32)
                nc.gpsimd.ap_gather(poolB[:], hcatB[:], idx128[0:80, :],
                                    channels=80, num_elems=SHARD, d=1,
                                    num_idxs=208)

                WA = qp.tile([128, K_TOP, 256], f32)
                nc.sync.dma_start(WA[:], WA_d[:])
                WB = qp.tile([80, K_TOP, 256], f32)
                nc.sync.dma_start(WB[:], WB_d[:])
                sc = qp.tile([128, 2], f32)
                nc.sync.dma_start(sc[:], sc_d[:])
                be = qp.tile([128, 2], f32)
                nc.sync.dma_start(be[:], be_d[:])
                Wc1s = qp.tile([128, 2, 128], f32)
                nc.sync.dma_start(Wc1s[:], Wc1_d[:])
                bc1 = qp.tile([128, 1], f32)
                nc.sync.dma_start(bc1[:], bc1_d[:])
                Wc2 = qp.tile([128, F], f32)
                nc.sync.dma_start(Wc2[:], Wc2_d[:])
                bc2 = qp.tile([F, 1], f32)
                nc.sync.dma_start(bc2[:], bc2_d[:])
                Wc3 = qp.tile([F, 1], f32)
                nc.sync.dma_start(Wc3[:], Wc3_d[:])
                bc3 = qp.tile([1, 1], f32)
                nc.sync.dma_start(bc3[:], bc3_d[:])

                z1 = []
                for mh in range(2):
                    pz = ps_w.tile([128, 13], f32, tag="pw", name=f"pz{mh}")
                    first = True
                    for r in range(K_TOP):
                        nc.tensor.matmul(
                            pz[:], WA[:, r, mh * 128:(mh + 1) * 128],
                            poolA[:, r:r + 16 * 12 + 1:16],
                            start=first, stop=False)
                        first = False
                        nc.tensor.matmul(
                            pz[:], WB[0:65, r, mh * 128:(mh + 1) * 128],
                            poolB[0:65, r:r + 16 * 12 + 1:16],
                            start=False, stop=(r == K_TOP - 1))
                    zz = qp.tile([128, 13], f32, tag=f"z1_{mh}", name=f"z1_{mh}")
                    nc.scalar.activation(zz[:], pz[:], AF.Relu,
                                         bias=be[:, mh:mh + 1],
                                         scale=sc[:, mh:mh + 1])
                    z1.append(zz)
                pz2 = ps_w.tile([128, 13], f32, tag="pw", name="pz2")
                nc.tensor.matmul(pz2[:], Wc1s[:, 0, :], z1[0][:],
                                 start=True, stop=False)
                nc.tensor.matmul(pz2[:], Wc1s[:, 1, :], z1[1][:],
                                 start=False, stop=True)
                z2 = qp.tile([128, 13], f32)
                nc.scalar.activation(z2[:], pz2[:], AF.Relu, bias=bc1[:])
                pz3 = ps_w.tile([F, 13], f32, tag="pw", name="pz3")
                nc.tensor.matmul(pz3[:], Wc2[:], z2[:], start=True, stop=True)
                z3 = qp.tile([F, 13], f32)
                nc.scalar.activation(z3[:], pz3[:], AF.Relu, bias=bc2[:])
                pz4 = ps_w.tile([1, 13], f32, tag="pw", name="pz4")
                nc.tensor.matmul(pz4[:], Wc3[:], z3[:], start=True, stop=True)
                zf = qp.tile([1, 13], f32)
                nc.vector.tensor_scalar(out=zf[:], in0=pz4[:],
                                        scalar1=bc3[0:1, 0:1], scalar2=None,
                                        op0=mybir.AluOpType.add)
                nc.sync.dma_start(out_d[:], zf[:])

    nc.compile()
    return nc


def kernel(**inputs):
    from concourse import bass_utils

    x = np.asarray(inputs["x"], np.float32)
    edge_index = np.asarray(inputs["edge_index"])

    key = ("prog",)
    CALLS, D, callbase, gidxs, u0bs, dinv_nms = _prep(x, edge_index)
    if key in _CACHE and _CACHE[key][0] == CALLS and np.array_equal(_CACHE[key][1], D):
        nc = _CACHE[key][2]
    else:
        nc = _build(CALLS, D, callbase)
        _CACHE[key] = (CALLS, D, nc)

    cw = _pack_classifier(inputs)
    in_maps = []
    for c in range(NCORES):
        m = {
            "u0b": u0bs[c],
            "gidx": gidxs[c],
            "dinv_nm": dinv_nms[c],
            "WA": cw["WA"], "WB": cw["WB"], "sc": cw["sc"], "be": cw["be"],
            "Wc1s": cw["Wc1s"], "bc1": cw["bc1"], "Wc2": cw["Wc2"],
            "bc2": cw["bc2"], "Wc3": cw["Wc3"], "bc3": cw["bc3"],
        }
        for i in range(4):
            m[f"W{i}"] = np.asarray(inputs[f"W{i}"], np.float32).reshape(
                F, F if i < 3 else 1)
            m[f"b{i}"] = np.asarray(inputs[f"b{i}"], np.float32).reshape(
                F if i < 3 else 1, 1)
        in_maps.append(m)

    trace = os.environ.get("KERNEL_TRACE", "0") == "1"
    kwargs = {}
    if trace:
        import sys, types
        if "antenv.axon_hooks" not in sys.modules:
            sys.path.insert(0, "/root/.axon_site")
            from trn_agent_boot.trn_boot import _ntff_profile_via_ctypes
            mm = types.ModuleType("antenv.axon_hooks")
            mm.get_axon_ntff_profile_hook = (
                lambda: _ntff_profile_via_ctypes("/opt/axon/libaxon_pjrt.so"))
            sys.modules["antenv.axon_hooks"] = mm
        import tempfile
        kwargs = dict(trace=True, tmpdir=tempfile.mkdtemp())

    res = bass_utils.run_bass_kernel_spmd(
        nc, in_maps, core_ids=list(range(NCORES)), **kwargs)

    global LAST_EXEC_NS
    LAST_EXEC_NS = res.exec_time_ns

    out = np.zeros((G, 1), np.float32)
    for c in range(NCORES):
        ngr = GRAPHS_PER_CORE[c]
        out[GSTART[c]:GSTART[c] + ngr, 0] = res.results[c]["out"][0, :ngr]
    return out


LAST_EXEC_NS = None


# revision 16
# speedup vs baseline: 1.3161x; 1.0363x over previous
"""DGCNN (4x GCNConv + sort-pool + MLP) on 8 trn2 NeuronCores.

Strategy: graph-parallel sharding (ranks 0-3: 13 graphs, 4-7: 12).
Interleaved slot layout (slot = per-graph degree rank * 13 + graph) so
each 128-dst round holds a narrow degree band across all graphs (fewer
padded slab columns than per-graph blocks) while the sort-pool reload
stays a pure affine DMA. Per layer: u = dinv*h staged node-major (u0
prebuilt on host) -> AllGather full f32 table -> per-slab-column
indirect row gathers into chunked SBUF tiles -> in-place vector-engine
tree reduction per round (+self-loop, dinv scale) -> PE transpose ->
feature transform + tanh per 4 rounds with inline staging of the next
layer's table. Sort-pool via max8/max_index/match_replace, pooled rows
extracted with ap_gather, classifier on PE.
"""
import os
import numpy as np

N = 50000
G = 100
NPG = 500
E = 800000
F = 64
K_TOP = 15
CAT = 193
NCORES = 8
SHARD = 6656
NTOT = SHARD * NCORES
NROUND = SHARD // 128  # 52
BN_EPS = 1e-5
ZROW = 41599           # core-0 pad slot 6655 in 3-piece table rows
CH_MAX = 112           # max gather columns per chunk

GRAPHS_PER_CORE = [13, 13, 13, 13, 12, 12, 12, 12]
GSTART = np.concatenate([[0], np.cumsum(GRAPHS_PER_CORE)])

_CACHE = {}


def _prep(x, edge_index):
    """Host-side sharding/index preprocessing. Pure numpy."""
    src = edge_index[0].astype(np.int64)
    dst = edge_index[1].astype(np.int64)

    deg = np.bincount(dst, minlength=N).astype(np.float32) + np.float32(1.0)
    dinv = deg ** np.float32(-0.5)
    indeg = np.bincount(dst, minlength=N).astype(np.int64)

    node_graph = np.arange(N) // NPG
    node_rank = np.searchsorted(GSTART, node_graph, side="right") - 1  # [N]

    # interleaved slot layout: per-graph degree rank (desc) r, local graph
    # g -> slot = r*13 + g. Rounds then hold narrow degree-quantile bands
    # across all graphs, and the pooling un-permute is a pure affine reload.
    slot_of = np.zeros(N, np.int64)
    for g in range(G):
        lo = g * NPG
        o = np.argsort(-indeg[lo:lo + NPG], kind="stable")
        r = np.empty(NPG, np.int64)
        r[o] = np.arange(NPG)
        g_local = g - GSTART[np.searchsorted(GSTART, g, side="right") - 1]
        slot_of[lo:lo + NPG] = r * 13 + g_local

    # 3-piece table rows (A: slots 0:3328, B: 3328:4992, C: 4992:6656),
    # each piece AllGather'd separately so staging can ship early pieces
    # while later rounds still gather. Row = base[p] + rank*rows[p] +
    # (slot - lo[p]).
    piece = ((slot_of >= 3328).astype(np.int64)
             + (slot_of >= 4992).astype(np.int64))
    pbase = np.array([0, 26624, 39936], np.int64)
    prows = np.array([3328, 1664, 1664], np.int64)
    plo = np.array([0, 3328, 4992], np.int64)
    pidx = pbase[piece] + node_rank * prows[piece] + (slot_of - plo[piece])

    # per-core edge lists grouped by dst slot
    e_rank = node_rank[dst]
    counts_all = np.zeros((NCORES, SHARD), np.int64)
    per_core = []
    for c in range(NCORES):
        m = e_rank == c
        s_p = pidx[src[m]].astype(np.int64)
        d_slot = slot_of[dst[m]]
        o = np.argsort(d_slot, kind="stable")
        s_p = s_p[o]
        d_slot = d_slot[o]
        cnt = np.bincount(d_slot, minlength=SHARD)
        counts_all[c] = cnt
        per_core.append((s_p, d_slot, cnt))

    D = counts_all.reshape(NCORES, NROUND, 128).max(axis=(0, 2))  # [NROUND]
    callbase = np.concatenate([[0], np.cumsum(D)])
    CALLS = int(callbase[-1])

    u0_full = np.zeros((NCORES * SHARD, F), np.float32)
    gidxs, u0bs, dinv_nms = [], [], []
    for c in range(NCORES):
        s_p, d_slot, cnt = per_core[c]
        off = np.concatenate([[0], np.cumsum(cnt)])
        gid = np.full((128, CALLS), ZROW, np.int32)
        j_within = np.arange(len(d_slot)) - off[d_slot]
        k = d_slot // 128
        p = d_slot % 128
        call = callbase[k] + j_within
        gid[p, call] = s_p.astype(np.int32)
        gidxs.append(gid)

        nodes = np.arange(NPG * GSTART[c], NPG * GSTART[c + 1])
        u0 = np.zeros((SHARD, F), np.float32)
        u0[slot_of[nodes]] = x[nodes] * dinv[nodes][:, None]
        u0bs.append(u0)
        for p in range(3):
            lo, rows = int(plo[p]), int(prows[p])
            u0_full[int(pbase[p]) + c * rows:
                    int(pbase[p]) + (c + 1) * rows] = u0[lo:lo + rows]

        dv = np.zeros(SHARD, np.float32)
        dv[slot_of[nodes]] = dinv[nodes]
        dinv_nms.append(np.ascontiguousarray(dv.reshape(NROUND, 128).T))

    return CALLS, D, callbase, gidxs, u0bs, dinv_nms, u0_full


def _chunks(D, callbase):
    """Round-aligned column chunks of at most CH_MAX columns."""
    out = []
    k0 = 0
    while k0 < NROUND:
        k1 = k0
        cols = 0
        while k1 < NROUND and cols + int(D[k1]) <= CH_MAX:
            cols += int(D[k1])
            k1 += 1
        if k1 == k0:  # single round exceeding CH_MAX (cannot happen: D<=128)
            k1 = k0 + 1
            cols = int(D[k0])
        out.append((k0, k1, int(callbase[k0]), int(callbase[k1])))
        k0 = k1
    k0, k1, c0, c1 = out[-1]
    if k1 - k0 > 3:  # short final chunk -> shorter post-gather tail
        cm = int(callbase[k1 - 2])
        out[-1] = (k0, k1 - 2, c0, cm)
        out.append((k1 - 2, k1, cm, c1))
    return out


def _pack_classifier(inp):
    Wc0 = np.asarray(inp["Wc0"], np.float32)           # [2895, 256]
    Wc0r = Wc0.reshape(K_TOP, CAT, 256)
    WA = np.ascontiguousarray(Wc0r[:, 0:128, :].transpose(1, 0, 2))   # [128,15,256]
    WB = np.zeros((80, K_TOP, 256), np.float32)
    WB[0:65] = Wc0r[:, 128:193, :].transpose(1, 0, 2)
    sc_full = np.asarray(inp["gamma"], np.float32) * np.float32(
        1.0 / np.sqrt(1.0 + BN_EPS))
    be_full = (np.asarray(inp["beta"], np.float32)
               + np.asarray(inp["bc0"], np.float32) * sc_full)
    sc = np.ascontiguousarray(sc_full.reshape(2, 128).T)  # [128, 2]
    be = np.ascontiguousarray(be_full.reshape(2, 128).T)
    Wc1 = np.asarray(inp["Wc1"], np.float32)              # [256, 128]
    Wc1s = np.ascontiguousarray(Wc1.reshape(2, 128, 128).transpose(1, 0, 2))
    return {
        "WA": WA, "WB": WB, "sc": sc, "be": be, "Wc1s": Wc1s,
        "bc1": np.asarray(inp["bc1"], np.float32).reshape(128, 1),
        "Wc2": np.asarray(inp["Wc2"], np.float32),
        "bc2": np.asarray(inp["bc2"], np.float32).reshape(64, 1),
        "Wc3": np.asarray(inp["Wc3"], np.float32),
        "bc3": np.asarray(inp["bc3"], np.float32).reshape(1, 1),
    }


def _build(CALLS, D, callbase):
    import concourse.bass as bass
    import concourse.bacc as bacc
    import concourse.mybir as mybir
    from concourse import tile
    from concourse.masks import make_identity

    f32 = mybir.dt.float32
    i32 = mybir.dt.int32
    AF = mybir.ActivationFunctionType
    chunks = _chunks(D, callbase)

    nc = bacc.Bacc("TRN2", target_bir_lowering=False, debug=False,
                   num_devices=NCORES)

    # ---- I/O ----
    u0b_d = nc.dram_tensor("u0b", [SHARD, F], f32, kind="ExternalInput")
    gidx_d = nc.dram_tensor("gidx", [128, CALLS], i32, kind="ExternalInput")
    dinv_d = nc.dram_tensor("dinv_nm", [128, NROUND], f32, kind="ExternalInput")
    W_d = [nc.dram_tensor(f"W{i}", [F, F if i < 3 else 1], f32,
                          kind="ExternalInput") for i in range(4)]
    b_d = [nc.dram_tensor(f"b{i}", [F if i < 3 else 1, 1], f32,
                          kind="ExternalInput") for i in range(4)]
    WA_d = nc.dram_tensor("WA", [128, K_TOP, 256], f32, kind="ExternalInput")
    WB_d = nc.dram_tensor("WB", [80, K_TOP, 256], f32, kind="ExternalInput")
    sc_d = nc.dram_tensor("sc", [128, 2], f32, kind="ExternalInput")
    be_d = nc.dram_tensor("be", [128, 2], f32, kind="ExternalInput")
    Wc1_d = nc.dram_tensor("Wc1s", [128, 2, 128], f32, kind="ExternalInput")
    bc1_d = nc.dram_tensor("bc1", [128, 1], f32, kind="ExternalInput")
    Wc2_d = nc.dram_tensor("Wc2", [128, F], f32, kind="ExternalInput")
    bc2_d = nc.dram_tensor("bc2", [F, 1], f32, kind="ExternalInput")
    Wc3_d = nc.dram_tensor("Wc3", [F, 1], f32, kind="ExternalInput")
    bc3_d = nc.dram_tensor("bc3", [1, 1], f32, kind="ExternalInput")
    out_d = nc.dram_tensor("out", [1, 13], f32, kind="ExternalOutput")

    u0full_d = nc.dram_tensor("u0_full", [NTOT, F], f32,
                              kind="ExternalInput")
    bounceA = nc.dram_tensor("bounceA", [3328, F], f32, kind="Internal")
    bounceB = nc.dram_tensor("bounceB", [1664, F], f32, kind="Internal")
    bounceC = nc.dram_tensor("bounceC", [1664, F], f32, kind="Internal")
    u_alls = [nc.dram_tensor(f"u_all{i}", [NTOT, F], f32, kind="Internal",
                             addr_space="Shared") for i in range(2)]
    d6656 = nc.dram_tensor("d6656", [1, SHARD], f32, kind="Internal")
    d208 = nc.dram_tensor("d208", [1, 208], mybir.dt.int16, kind="Internal")

    with tile.TileContext(nc) as tc:
        with (
            tc.tile_pool(name="persist", bufs=1) as pp,
            tc.tile_pool(name="psum_t", bufs=4, space="PSUM") as ps_t,
            tc.tile_pool(name="psum_w", bufs=2, space="PSUM") as ps_w,
        ):
            ident = pp.tile([128, 128], f32)
            make_identity(nc, ident[:])
            gidx = pp.tile([128, CALLS], i32)
            nc.sync.dma_start(gidx[:], gidx_d[:])
            dinv = pp.tile([128, NROUND], f32)
            nc.sync.dma_start(dinv[:], dinv_d[:])
            Ws, bs = [], []
            for i in range(4):
                w = pp.tile([F, F if i < 3 else 1], f32, name=f"W{i}s")
                nc.sync.dma_start(w[:], W_d[i][:])
                Ws.append(w)
                bb = pp.tile([F if i < 3 else 1, 1], f32, name=f"b{i}s")
                nc.sync.dma_start(bb[:], b_d[i][:])
                bs.append(bb)
            hcatA = pp.tile([128, SHARD], f32)   # h0 (rows 0:64), h1 (64:128)
            hcatB = pp.tile([80, SHARD], f32)    # h2 (0:64), h3 (row 64)

            with tc.tile_pool(name="conv", bufs=1) as cp:
                u_nm = cp.tile([128, NROUND, F], f32, tag="u_nm",
                               name="u_nm0", bufs=2)
                nc.sync.dma_start(
                    u_nm[:], u0b_d.ap().rearrange("(t p) f -> p t f", p=128))
                def coll_piece(dst, which):
                    src = (bounceA, bounceB, bounceC)[which]
                    base = (0, 26624, 39936)[which]
                    rows = (3328, 1664, 1664)[which]
                    nc.gpsimd.collective_compute(
                        "AllGather", mybir.AluOpType.bypass,
                        replica_groups=[list(range(NCORES))],
                        ins=[src.ap()],
                        outs=[dst.ap()[base:base + NCORES * rows, :]])

                # layer-0 table comes prebuilt from the host; collectives for
                # layer l+1 write the alternate table while layer l gathers.
                tabs = [u0full_d, u_alls[1], u_alls[0], u_alls[1]]
                for layer in range(4):
                    u_all = tabs[layer]

                    fo = F if layer < 3 else 1
                    hout = (hcatA[0:64, :] if layer == 0 else
                            hcatA[64:128, :] if layer == 1 else
                            hcatB[0:64, :] if layer == 2 else
                            hcatB[64:65, :])
                    if layer < 3:
                        u_next = cp.tile([128, NROUND, F], f32, tag="u_nm",
                                         name=f"u_nm{layer + 1}", bufs=2)
                    s_fm = cp.tile([F, SHARD], f32, tag="s_fm",
                                   name=f"s_fm{layer}", bufs=1)

                    collA_issued = 0
                    for (k0, k1, c0, c1) in chunks:
                        cols = c1 - c0
                        gt = cp.tile([128, CH_MAX, F], f32, tag="gt",
                                     name=f"gt{layer}_{k0}", bufs=2)
                        for c in range(c0, c1):
                            nc.gpsimd.indirect_dma_start(
                                out=gt[:, c - c0, :], out_offset=None,
                                in_=u_all[:],
                                in_offset=bass.IndirectOffsetOnAxis(
                                    ap=gidx[:, c:c + 1], axis=0))
                        for k in range(k0, k1):
                            dk = int(D[k])
                            q0 = int(callbase[k]) - c0
                            # in-place tree reduction of dk slab columns
                            d_cur = dk
                            while d_cur > 1:
                                h = d_cur // 2
                                nc.vector.tensor_tensor(
                                    out=gt[:, q0:q0 + h, :],
                                    in0=gt[:, q0:q0 + h, :],
                                    in1=gt[:, q0 + d_cur - h:q0 + d_cur, :],
                                    op=mybir.AluOpType.add)
                                d_cur -= h
                            s_nm = cp.tile([128, F], f32, tag="s_nm",
                                           name=f"s_nm{layer}_{k}", bufs=4)
                            if dk > 0:
                                nc.vector.tensor_tensor(
                                    out=s_nm[:], in0=gt[:, q0, :],
                                    in1=u_nm[:, k, :],
                                    op=mybir.AluOpType.add)
                            else:
                                nc.vector.tensor_copy(s_nm[:], u_nm[:, k, :])
                            nc.vector.tensor_tensor(
                                out=s_nm[:], in0=s_nm[:],
                                in1=dinv[:, k:k + 1].to_broadcast([128, F]),
                                op=mybir.AluOpType.mult)
                            # transpose to feature-major
                            ptile = ps_t.tile([F, 128], f32, tag="trf",
                                              name=f"ptf{layer}_{k}", bufs=2)
                            nc.tensor.matmul(ptile[:], s_nm[:], ident[:],
                                             is_transpose=True)
                            nc.scalar.activation(
                                s_fm[:, k * 128:(k + 1) * 128], ptile[:],
                                AF.Copy)
                            # transform + tanh + inline staging per 4 rounds
                            if k % 4 == 3 or k == NROUND - 1:
                                t0 = (k // 4) * 4
                                nc0 = t0 * 128
                                nc1 = (k + 1) * 128
                                pw = ps_w.tile([fo, 512], f32, tag="pw",
                                               name=f"pw{layer}_{t0}")
                                nc.tensor.matmul(pw[:, 0:nc1 - nc0],
                                                 Ws[layer][:],
                                                 s_fm[:, nc0:nc1],
                                                 start=True, stop=True)
                                nc.scalar.activation(
                                    hout[:, nc0:nc1], pw[:, 0:nc1 - nc0],
                                    AF.Tanh, bias=bs[layer][:])
                                if layer < 3:
                                    for t in range(t0, k + 1):
                                        pt2 = ps_t.tile(
                                            [128, F], f32, tag="trs",
                                            name=f"pts{layer}_{t}", bufs=2)
                                        ident64 = (ident[64:128, 64:128]
                                                   if layer == 1
                                                   else ident[0:F, 0:F])
                                        nc.tensor.matmul(
                                            pt2[:],
                                            hout[:, t * 128:(t + 1) * 128],
                                            ident64,
                                            is_transpose=True)
                                        nc.vector.tensor_tensor(
                                            out=u_next[:, t, :], in0=pt2[:],
                                            in1=dinv[:, t:t + 1]
                                            .to_broadcast([128, F]),
                                            op=mybir.AluOpType.mult)
                        if layer < 3 and collA_issued == 0 and k1 >= 28:
                            # rounds 0-27 staged: ship piece A of next table
                            # while the rest of this layer still gathers.
                            nc.sync.dma_start(
                                bounceA.ap().rearrange(
                                    "(t p) f -> p t f", p=128),
                                u_next[:, 0:26, :])
                            coll_piece(u_alls[(layer + 1) % 2], 0)
                            collA_issued = 1
                        if layer < 3 and collA_issued == 1 and k1 >= 40:
                            nc.sync.dma_start(
                                bounceB.ap().rearrange(
                                    "(t p) f -> p t f", p=128),
                                u_next[:, 26:39, :])
                            coll_piece(u_alls[(layer + 1) % 2], 1)
                            collA_issued = 2
                    if layer < 3:
                        nc.sync.dma_start(
                            bounceC.ap().rearrange("(t p) f -> p t f", p=128),
                            u_next[:, 39:NROUND, :])
                        coll_piece(u_alls[(layer + 1) % 2], 2)
                        u_nm = u_next

            # ================= sort-pool + classifier =================
            with tc.tile_pool(name="poolp", bufs=1) as qp:
                h3 = hcatB[64:65, :]
                nc.sync.dma_start(d6656.ap(), h3)
                h3gt = qp.tile([13, NPG], f32)
                nc.sync.dma_start(
                    h3gt[:],
                    d6656.ap()[:, 0:6500].rearrange(
                        "one (i g) -> (one g) i", g=13))
                h3g = h3gt[:]

                m8a = qp.tile([13, 8], f32)
                i8a = qp.tile([13, 8], mybir.dt.uint32)
                nc.vector.max(m8a[:], h3g)
                nc.vector.max_index(i8a[:], m8a[:], h3g)
                h3m = qp.tile([13, NPG], f32)
                nc.vector.match_replace(h3m[:], m8a[:], h3g, -2.0)
                m8b = qp.tile([13, 8], f32)
                i8b = qp.tile([13, 8], mybir.dt.uint32)
                nc.vector.max(m8b[:], h3m[:])
                nc.vector.max_index(i8b[:], m8b[:], h3m[:])

                idx2d = qp.tile([13, 16], f32)
                nc.vector.memset(idx2d[:], 0.0)
                nc.vector.tensor_copy(idx2d[:, 0:8], i8a[:])
                nc.vector.tensor_copy(idx2d[:, 8:15], i8b[:, 0:7])
                gof = qp.tile([13, 1], mybir.dt.int32)
                nc.gpsimd.iota(gof[:], [[0, 1]], base=0, channel_multiplier=1)
                goff = qp.tile([13, 1], f32)
                nc.vector.tensor_copy(goff[:], gof[:])
                # absolute slot = pos*13 + g
                nc.vector.tensor_scalar(
                    out=idx2d[:], in0=idx2d[:], scalar1=13.0,
                    scalar2=None, op0=mybir.AluOpType.mult)
                nc.vector.tensor_scalar(
                    out=idx2d[:], in0=idx2d[:], scalar1=goff[:, 0:1],
                    scalar2=None, op0=mybir.AluOpType.add)
                idx16 = qp.tile([13, 16], mybir.dt.int16)
                nc.vector.tensor_copy(idx16[:], idx2d[:])
                nc.sync.dma_start(
                    d208.ap().rearrange("one (g r) -> (one g) r", g=13),
                    idx16[:])
                idx128 = qp.tile([128, 13], mybir.dt.int16)
                for kk in range(8):
                    nc.sync.dma_start(
                        idx128[kk * 16:(kk + 1) * 16, :],
                        d208.ap().rearrange("one (s p) -> (one p) s", p=16))

                poolA = qp.tile([128, 208], f32)
                nc.gpsimd.ap_gather(poolA[:], hcatA[:], idx128[:],
                                    channels=128, num_elems=SHARD, d=1,
                                    num_idxs=208)
                poolB = qp.tile([80, 208], f32)
                nc.gpsimd.ap_gather(poolB[:], hcatB[:], idx128[0:80, :],
                                    channels=80, num_elems=SHARD, d=1,
                                    num_idxs=208)

                WA = qp.tile([128, K_TOP, 256], f32)
                nc.sync.dma_start(WA[:], WA_d[:])
                WB = qp.tile([80, K_TOP, 256], f32)
                nc.sync.dma_start(WB[:], WB_d[:])
                sc = qp.tile([128, 2], f32)
                nc.sync.dma_start(sc[:], sc_d[:])
                be = qp.tile([128, 2], f32)
                nc.sync.dma_start(be[:], be_d[:])
                Wc1s = qp.tile([128, 2, 128], f32)
                nc.sync.dma_start(Wc1s[:], Wc1_d[:])
                bc1 = qp.tile([128, 1], f32)
                nc.sync.dma_start(bc1[:], bc1_d[:])
                Wc2 = qp.tile([128, F], f32)
                nc.sync.dma_start(Wc2[:], Wc2_d[:])
                bc2 = qp.tile([F, 1], f32)
                nc.sync.dma_start(bc2[:], bc2_d[:])
                Wc3 = qp.tile([F, 1], f32)
                nc.sync.dma_start(Wc3[:], Wc3_d[:])
                bc3 = qp.tile([1, 1], f32)
                nc.sync.dma_start(bc3[:], bc3_d[:])

                z1 = []
                for mh in range(2):
                    pz = ps_w.tile([128, 13], f32, tag="pw", name=f"pz{mh}")
                    first = True
                    for r in range(K_TOP):
                        nc.tensor.matmul(
                            pz[:], WA[:, r, mh * 128:(mh + 1) * 128],
                            poolA[:, r:r + 16 * 12 + 1:16],
                            start=first, stop=False)
                        first = False
                        nc.tensor.matmul(
                            pz[:], WB[0:65, r, mh * 128:(mh + 1) * 128],
                            poolB[0:65, r:r + 16 * 12 + 1:16],
                            start=False, stop=(r == K_TOP - 1))
                    zz = qp.tile([128, 13], f32, tag=f"z1_{mh}", name=f"z1_{mh}")
                    nc.scalar.activation(zz[:], pz[:], AF.Relu,
                                         bias=be[:, mh:mh + 1],
                                         scale=sc[:, mh:mh + 1])
                    z1.append(zz)
                pz2 = ps_w.tile([128, 13], f32, tag="pw", name="pz2")
                nc.tensor.matmul(pz2[:], Wc1s[:, 0, :], z1[0][:],
                                 start=True, stop=False)
                nc.tensor.matmul(pz2[:], Wc1s[:, 1, :], z1[1][:],
                                 start=False, stop=True)
                z2 = qp.tile([128, 13], f32)
                nc.scalar.activation(z2[:], pz2[:], AF.Relu, bias=bc1[:])
                pz3 = ps_w.tile([F, 13], f32, tag="pw", name="pz3")
                nc.tensor.matmul(pz3[:], Wc2[:], z2[:], start=True, stop=True)
                z3 = qp.tile([F, 13], f32)
                nc.scalar.activation(z3[:], pz3[:], AF.Relu, bias=bc2[:])
                pz4 = ps_w.tile([1, 13], f32, tag="pw", name="pz4")
                nc.tensor.matmul(pz4[:], Wc3[:], z3[:], start=True, stop=True)
                zf = qp.tile([1, 13], f32)
                nc.vector.tensor_scalar(out=zf[:], in0=pz4[:],
                                        scalar1=bc3[0:1, 0:1], scalar2=None,
                                        op0=mybir.AluOpType.add)
                nc.sync.dma_start(out_d[:], zf[:])

    nc.compile()
    return nc


def kernel(**inputs):
    from concourse import bass_utils

    x = np.asarray(inputs["x"], np.float32)
    edge_index = np.asarray(inputs["edge_index"])

    key = ("prog",)
    CALLS, D, callbase, gidxs, u0bs, dinv_nms, u0_full = _prep(x, edge_index)
    if key in _CACHE and _CACHE[key][0] == CALLS and np.array_equal(_CACHE[key][1], D):
        nc = _CACHE[key][2]
    else:
        nc = _build(CALLS, D, callbase)
        _CACHE[key] = (CALLS, D, nc)

    cw = _pack_classifier(inputs)
    in_maps = []
    for c in range(NCORES):
        m = {
            "u0b": u0bs[c],
            "u0_full": u0_full,
            "gidx": gidxs[c],
            "dinv_nm": dinv_nms[c],
            "WA": cw["WA"], "WB": cw["WB"], "sc": cw["sc"], "be": cw["be"],
            "Wc1s": cw["Wc1s"], "bc1": cw["bc1"], "Wc2": cw["Wc2"],
            "bc2": cw["bc2"], "Wc3": cw["Wc3"], "bc3": cw["bc3"],
        }
        for i in range(4):
            m[f"W{i}"] = np.asarray(inputs[f"W{i}"], np.float32).reshape(
                F, F if i < 3 else 1)
            m[f"b{i}"] = np.asarray(inputs[f"b{i}"], np.float32).reshape(
                F if i < 3 else 1, 1)
        in_maps.append(m)

    trace = os.environ.get("KERNEL_TRACE", "0") == "1"
    kwargs = {}
    if trace:
        import sys, types
        if "antenv.axon_hooks" not in sys.modules:
            sys.path.insert(0, "/root/.axon_site")
            from trn_agent_boot.trn_boot import _ntff_profile_via_ctypes
            mm = types.ModuleType("antenv.axon_hooks")
            mm.get_axon_ntff_profile_hook = (
                lambda: _ntff_profile_via_ctypes("/opt/axon/libaxon_pjrt.so"))
            sys.modules["antenv.axon_hooks"] = mm
        import tempfile
        kwargs = dict(trace=True, tmpdir=tempfile.mkdtemp())

    res = bass_utils.run_bass_kernel_spmd(
        nc, in_maps, core_ids=list(range(NCORES)), **kwargs)

    global LAST_EXEC_NS
    LAST_EXEC_NS = res.exec_time_ns

    out = np.zeros((G, 1), np.float32)
    for c in range(NCORES):
        ngr = GRAPHS_PER_CORE[c]
        out[GSTART[c]:GSTART[c] + ngr, 0] = res.results[c]["out"][0, :ngr]
    return out


LAST_EXEC_NS = None


# revision 17
# speedup vs baseline: 1.3167x; 1.0004x over previous
"""DGCNN (4x GCNConv + sort-pool + MLP) on 8 trn2 NeuronCores.

Strategy: graph-parallel sharding (ranks 0-3: 13 graphs, 4-7: 12).
Interleaved slot layout (slot = per-graph degree rank * 13 + graph) so
each 128-dst round holds a narrow degree band across all graphs (fewer
padded slab columns than per-graph blocks) while the sort-pool reload
stays a pure affine DMA. Per layer: u = dinv*h staged node-major (u0
prebuilt on host) -> AllGather full f32 table -> per-slab-column
indirect row gathers into chunked SBUF tiles -> in-place vector-engine
tree reduction per round (+self-loop, dinv scale) -> PE transpose ->
feature transform + tanh per 4 rounds with inline staging of the next
layer's table. Sort-pool via max8/max_index/match_replace, pooled rows
extracted with ap_gather, classifier on PE.
"""
import os
import numpy as np

N = 50000
G = 100
NPG = 500
E = 800000
F = 64
K_TOP = 15
CAT = 193
NCORES = 8
SHARD = 6656
NTOT = SHARD * NCORES
NROUND = SHARD // 128  # 52
BN_EPS = 1e-5
ZROW = 41599           # core-0 pad slot 6655 in 3-piece table rows
CH_MAX = 112           # max gather columns per chunk

GRAPHS_PER_CORE = [13, 13, 13, 13, 12, 12, 12, 12]
GSTART = np.concatenate([[0], np.cumsum(GRAPHS_PER_CORE)])

_CACHE = {}


def _prep(x, edge_index):
    """Host-side sharding/index preprocessing. Pure numpy."""
    src = edge_index[0].astype(np.int64)
    dst = edge_index[1].astype(np.int64)

    deg = np.bincount(dst, minlength=N).astype(np.float32) + np.float32(1.0)
    dinv = deg ** np.float32(-0.5)
    indeg = np.bincount(dst, minlength=N).astype(np.int64)

    node_graph = np.arange(N) // NPG
    node_rank = np.searchsorted(GSTART, node_graph, side="right") - 1  # [N]

    # interleaved slot layout: per-graph degree rank (desc) r, local graph
    # g -> slot = r*13 + g. Rounds then hold narrow degree-quantile bands
    # across all graphs, and the pooling un-permute is a pure affine reload.
    slot_of = np.zeros(N, np.int64)
    for g in range(G):
        lo = g * NPG
        o = np.argsort(-indeg[lo:lo + NPG], kind="stable")
        r = np.empty(NPG, np.int64)
        r[o] = np.arange(NPG)
        g_local = g - GSTART[np.searchsorted(GSTART, g, side="right") - 1]
        slot_of[lo:lo + NPG] = r * 13 + g_local

    # 3-piece table rows (A: slots 0:3328, B: 3328:4992, C: 4992:6656),
    # each piece AllGather'd separately so staging can ship early pieces
    # while later rounds still gather. Row = base[p] + rank*rows[p] +
    # (slot - lo[p]).
    piece = ((slot_of >= 3328).astype(np.int64)
             + (slot_of >= 4992).astype(np.int64))
    pbase = np.array([0, 26624, 39936], np.int64)
    prows = np.array([3328, 1664, 1664], np.int64)
    plo = np.array([0, 3328, 4992], np.int64)
    pidx = pbase[piece] + node_rank * prows[piece] + (slot_of - plo[piece])

    # per-core edge lists grouped by dst slot
    e_rank = node_rank[dst]
    counts_all = np.zeros((NCORES, SHARD), np.int64)
    per_core = []
    for c in range(NCORES):
        m = e_rank == c
        s_p = pidx[src[m]].astype(np.int64)
        d_slot = slot_of[dst[m]]
        o = np.argsort(d_slot, kind="stable")
        s_p = s_p[o]
        d_slot = d_slot[o]
        cnt = np.bincount(d_slot, minlength=SHARD)
        counts_all[c] = cnt
        per_core.append((s_p, d_slot, cnt))

    D = counts_all.reshape(NCORES, NROUND, 128).max(axis=(0, 2))  # [NROUND]
    callbase = np.concatenate([[0], np.cumsum(D)])
    CALLS = int(callbase[-1])

    u0_full = np.zeros((NCORES * SHARD, F), np.float32)
    gidxs, u0bs, dinv_nms = [], [], []
    for c in range(NCORES):
        s_p, d_slot, cnt = per_core[c]
        off = np.concatenate([[0], np.cumsum(cnt)])
        gid = np.full((128, CALLS), ZROW, np.int32)
        j_within = np.arange(len(d_slot)) - off[d_slot]
        k = d_slot // 128
        p = d_slot % 128
        call = callbase[k] + j_within
        gid[p, call] = s_p.astype(np.int32)
        gidxs.append(gid)

        nodes = np.arange(NPG * GSTART[c], NPG * GSTART[c + 1])
        u0 = np.zeros((SHARD, F), np.float32)
        u0[slot_of[nodes]] = x[nodes] * dinv[nodes][:, None]
        u0bs.append(u0)
        for p in range(3):
            lo, rows = int(plo[p]), int(prows[p])
            u0_full[int(pbase[p]) + c * rows:
                    int(pbase[p]) + (c + 1) * rows] = u0[lo:lo + rows]

        dv = np.zeros(SHARD, np.float32)
        dv[slot_of[nodes]] = dinv[nodes]
        dinv_nms.append(np.ascontiguousarray(dv.reshape(NROUND, 128).T))

    return CALLS, D, callbase, gidxs, u0bs, dinv_nms, u0_full


def _chunks(D, callbase):
    """Round-aligned column chunks of at most CH_MAX columns."""
    out = []
    k0 = 0
    while k0 < NROUND:
        k1 = k0
        cols = 0
        while k1 < NROUND and cols + int(D[k1]) <= CH_MAX:
            cols += int(D[k1])
            k1 += 1
        if k1 == k0:  # single round exceeding CH_MAX (cannot happen: D<=128)
            k1 = k0 + 1
            cols = int(D[k0])
        out.append((k0, k1, int(callbase[k0]), int(callbase[k1])))
        k0 = k1
    k0, k1, c0, c1 = out[-1]
    if k1 - k0 > 3:  # short final chunk -> shorter post-gather tail
        cm = int(callbase[k1 - 2])
        out[-1] = (k0, k1 - 2, c0, cm)
        out.append((k1 - 2, k1, cm, c1))
    return out


def _pack_classifier(inp):
    Wc0 = np.asarray(inp["Wc0"], np.float32)           # [2895, 256]
    Wc0r = Wc0.reshape(K_TOP, CAT, 256)
    WA = np.ascontiguousarray(Wc0r[:, 0:128, :].transpose(1, 0, 2))   # [128,15,256]
    WB = np.zeros((80, K_TOP, 256), np.float32)
    WB[0:65] = Wc0r[:, 128:193, :].transpose(1, 0, 2)
    sc_full = np.asarray(inp["gamma"], np.float32) * np.float32(
        1.0 / np.sqrt(1.0 + BN_EPS))
    be_full = (np.asarray(inp["beta"], np.float32)
               + np.asarray(inp["bc0"], np.float32) * sc_full)
    sc = np.ascontiguousarray(sc_full.reshape(2, 128).T)  # [128, 2]
    be = np.ascontiguousarray(be_full.reshape(2, 128).T)
    Wc1 = np.asarray(inp["Wc1"], np.float32)              # [256, 128]
    Wc1s = np.ascontiguousarray(Wc1.reshape(2, 128, 128).transpose(1, 0, 2))
    return {
        "WA": WA, "WB": WB, "sc": sc, "be": be, "Wc1s": Wc1s,
        "bc1": np.asarray(inp["bc1"], np.float32).reshape(128, 1),
        "Wc2": np.asarray(inp["Wc2"], np.float32),
        "bc2": np.asarray(inp["bc2"], np.float32).reshape(64, 1),
        "Wc3": np.asarray(inp["Wc3"], np.float32),
        "bc3": np.asarray(inp["bc3"], np.float32).reshape(1, 1),
    }


def _build(CALLS, D, callbase):
    import concourse.bass as bass
    import concourse.bacc as bacc
    import concourse.mybir as mybir
    from concourse import tile
    from concourse.masks import make_identity

    f32 = mybir.dt.float32
    i32 = mybir.dt.int32
    AF = mybir.ActivationFunctionType
    chunks = _chunks(D, callbase)

    nc = bacc.Bacc("TRN2", target_bir_lowering=False, debug=False,
                   num_devices=NCORES)

    # ---- I/O ----
    u0b_d = nc.dram_tensor("u0b", [SHARD, F], f32, kind="ExternalInput")
    gidx_d = nc.dram_tensor("gidx", [128, CALLS], i32, kind="ExternalInput")
    dinv_d = nc.dram_tensor("dinv_nm", [128, NROUND], f32, kind="ExternalInput")
    W_d = [nc.dram_tensor(f"W{i}", [F, F if i < 3 else 1], f32,
                          kind="ExternalInput") for i in range(4)]
    b_d = [nc.dram_tensor(f"b{i}", [F if i < 3 else 1, 1], f32,
                          kind="ExternalInput") for i in range(4)]
    WA_d = nc.dram_tensor("WA", [128, K_TOP, 256], f32, kind="ExternalInput")
    WB_d = nc.dram_tensor("WB", [80, K_TOP, 256], f32, kind="ExternalInput")
    sc_d = nc.dram_tensor("sc", [128, 2], f32, kind="ExternalInput")
    be_d = nc.dram_tensor("be", [128, 2], f32, kind="ExternalInput")
    Wc1_d = nc.dram_tensor("Wc1s", [128, 2, 128], f32, kind="ExternalInput")
    bc1_d = nc.dram_tensor("bc1", [128, 1], f32, kind="ExternalInput")
    Wc2_d = nc.dram_tensor("Wc2", [128, F], f32, kind="ExternalInput")
    bc2_d = nc.dram_tensor("bc2", [F, 1], f32, kind="ExternalInput")
    Wc3_d = nc.dram_tensor("Wc3", [F, 1], f32, kind="ExternalInput")
    bc3_d = nc.dram_tensor("bc3", [1, 1], f32, kind="ExternalInput")
    out_d = nc.dram_tensor("out", [1, 13], f32, kind="ExternalOutput")

    u0full_d = nc.dram_tensor("u0_full", [NTOT, F], f32,
                              kind="ExternalInput")
    bounceA = nc.dram_tensor("bounceA", [3328, F], f32, kind="Internal")
    bounceB = nc.dram_tensor("bounceB", [1664, F], f32, kind="Internal")
    bounceC = nc.dram_tensor("bounceC", [1664, F], f32, kind="Internal")
    u_alls = [nc.dram_tensor(f"u_all{i}", [NTOT, F], f32, kind="Internal",
                             addr_space="Shared") for i in range(2)]
    d6656 = nc.dram_tensor("d6656", [1, SHARD], f32, kind="Internal")
    d208 = nc.dram_tensor("d208", [1, 208], mybir.dt.int16, kind="Internal")

    with tile.TileContext(nc) as tc:
        with (
            tc.tile_pool(name="persist", bufs=1) as pp,
            tc.tile_pool(name="psum_t", bufs=4, space="PSUM") as ps_t,
            tc.tile_pool(name="psum_w", bufs=2, space="PSUM") as ps_w,
        ):
            ident = pp.tile([128, 128], f32)
            make_identity(nc, ident[:])
            gidx = pp.tile([128, CALLS], i32)
            nc.sync.dma_start(gidx[:], gidx_d[:])
            dinv = pp.tile([128, NROUND], f32)
            nc.sync.dma_start(dinv[:], dinv_d[:])
            Ws, bs = [], []
            for i in range(4):
                w = pp.tile([F, F if i < 3 else 1], f32, name=f"W{i}s")
                nc.sync.dma_start(w[:], W_d[i][:])
                Ws.append(w)
                bb = pp.tile([F if i < 3 else 1, 1], f32, name=f"b{i}s")
                nc.sync.dma_start(bb[:], b_d[i][:])
                bs.append(bb)
            hcatA = pp.tile([128, SHARD], f32)   # h0 (rows 0:64), h1 (64:128)
            hcatB = pp.tile([80, SHARD], f32)    # h2 (0:64), h3 (row 64)
            WA = pp.tile([128, K_TOP, 256], f32)
            nc.sync.dma_start(WA[:], WA_d[:])
            WB = pp.tile([80, K_TOP, 256], f32)
            nc.sync.dma_start(WB[:], WB_d[:])
            sc = pp.tile([128, 2], f32)
            nc.sync.dma_start(sc[:], sc_d[:])
            be = pp.tile([128, 2], f32)
            nc.sync.dma_start(be[:], be_d[:])
            Wc1s = pp.tile([128, 2, 128], f32)
            nc.sync.dma_start(Wc1s[:], Wc1_d[:])
            bc1 = pp.tile([128, 1], f32)
            nc.sync.dma_start(bc1[:], bc1_d[:])
            Wc2 = pp.tile([128, F], f32)
            nc.sync.dma_start(Wc2[:], Wc2_d[:])
            bc2 = pp.tile([F, 1], f32)
            nc.sync.dma_start(bc2[:], bc2_d[:])
            Wc3 = pp.tile([F, 1], f32)
            nc.sync.dma_start(Wc3[:], Wc3_d[:])
            bc3 = pp.tile([1, 1], f32)
            nc.sync.dma_start(bc3[:], bc3_d[:])

            with tc.tile_pool(name="conv", bufs=1) as cp:
                u_nm = cp.tile([128, NROUND, F], f32, tag="u_nm",
                               name="u_nm0", bufs=2)
                nc.sync.dma_start(
                    u_nm[:], u0b_d.ap().rearrange("(t p) f -> p t f", p=128))
                def coll_piece(dst, which):
                    src = (bounceA, bounceB, bounceC)[which]
                    base = (0, 26624, 39936)[which]
                    rows = (3328, 1664, 1664)[which]
                    nc.gpsimd.collective_compute(
                        "AllGather", mybir.AluOpType.bypass,
                        replica_groups=[list(range(NCORES))],
                        ins=[src.ap()],
                        outs=[dst.ap()[base:base + NCORES * rows, :]])

                # layer-0 table comes prebuilt from the host; collectives for
                # layer l+1 write the alternate table while layer l gathers.
                tabs = [u0full_d, u_alls[1], u_alls[0], u_alls[1]]
                for layer in range(4):
                    u_all = tabs[layer]

                    fo = F if layer < 3 else 1
                    hout = (hcatA[0:64, :] if layer == 0 else
                            hcatA[64:128, :] if layer == 1 else
                            hcatB[0:64, :] if layer == 2 else
                            hcatB[64:65, :])
                    if layer < 3:
                        u_next = cp.tile([128, NROUND, F], f32, tag="u_nm",
                                         name=f"u_nm{layer + 1}", bufs=2)
                    s_fm = cp.tile([F, SHARD], f32, tag="s_fm",
                                   name=f"s_fm{layer}", bufs=1)

                    collA_issued = 0
                    for (k0, k1, c0, c1) in chunks:
                        cols = c1 - c0
                        gt = cp.tile([128, CH_MAX, F], f32, tag="gt",
                                     name=f"gt{layer}_{k0}", bufs=2)
                        for c in range(c0, c1):
                            nc.gpsimd.indirect_dma_start(
                                out=gt[:, c - c0, :], out_offset=None,
                                in_=u_all[:],
                                in_offset=bass.IndirectOffsetOnAxis(
                                    ap=gidx[:, c:c + 1], axis=0))
                        for k in range(k0, k1):
                            dk = int(D[k])
                            q0 = int(callbase[k]) - c0
                            # in-place tree reduction of dk slab columns
                            d_cur = dk
                            while d_cur > 1:
                                h = d_cur // 2
                                nc.vector.tensor_tensor(
                                    out=gt[:, q0:q0 + h, :],
                                    in0=gt[:, q0:q0 + h, :],
                                    in1=gt[:, q0 + d_cur - h:q0 + d_cur, :],
                                    op=mybir.AluOpType.add)
                                d_cur -= h
                            s_nm = cp.tile([128, F], f32, tag="s_nm",
                                           name=f"s_nm{layer}_{k}", bufs=4)
                            if dk > 0:
                                nc.vector.tensor_tensor(
                                    out=s_nm[:], in0=gt[:, q0, :],
                                    in1=u_nm[:, k, :],
                                    op=mybir.AluOpType.add)
                            else:
                                nc.vector.tensor_copy(s_nm[:], u_nm[:, k, :])
                            nc.vector.tensor_tensor(
                                out=s_nm[:], in0=s_nm[:],
                                in1=dinv[:, k:k + 1].to_broadcast([128, F]),
                                op=mybir.AluOpType.mult)
                            # transpose to feature-major
                            ptile = ps_t.tile([F, 128], f32, tag="trf",
                                              name=f"ptf{layer}_{k}", bufs=2)
                            nc.tensor.matmul(ptile[:], s_nm[:], ident[:],
                                             is_transpose=True)
                            nc.scalar.activation(
                                s_fm[:, k * 128:(k + 1) * 128], ptile[:],
                                AF.Copy)
                            # transform + tanh + inline staging per 4 rounds
                            if k % 4 == 3 or k == NROUND - 1:
                                t0 = (k // 4) * 4
                                nc0 = t0 * 128
                                nc1 = (k + 1) * 128
                                pw = ps_w.tile([fo, 512], f32, tag="pw",
                                               name=f"pw{layer}_{t0}")
                                nc.tensor.matmul(pw[:, 0:nc1 - nc0],
                                                 Ws[layer][:],
                                                 s_fm[:, nc0:nc1],
                                                 start=True, stop=True)
                                nc.scalar.activation(
                                    hout[:, nc0:nc1], pw[:, 0:nc1 - nc0],
                                    AF.Tanh, bias=bs[layer][:])
                                if layer < 3:
                                    for t in range(t0, k + 1):
                                        pt2 = ps_t.tile(
                                            [128, F], f32, tag="trs",
                                            name=f"pts{layer}_{t}", bufs=2)
                                        ident64 = (ident[64:128, 64:128]
                                                   if layer == 1
                                                   else ident[0:F, 0:F])
                                        nc.tensor.matmul(
                                            pt2[:],
                                            hout[:, t * 128:(t + 1) * 128],
                                            ident64,
                                            is_transpose=True)
                                        nc.vector.tensor_tensor(
                                            out=u_next[:, t, :], in0=pt2[:],
                                            in1=dinv[:, t:t + 1]
                                            .to_broadcast([128, F]),
                                            op=mybir.AluOpType.mult)
                        if layer < 3 and collA_issued == 0 and k1 >= 28:
                            # rounds 0-27 staged: ship piece A of next table
                            # while the rest of this layer still gathers.
                            nc.sync.dma_start(
                                bounceA.ap().rearrange(
                                    "(t p) f -> p t f", p=128),
                                u_next[:, 0:26, :])
                            coll_piece(u_alls[(layer + 1) % 2], 0)
                            collA_issued = 1
                        if layer < 3 and collA_issued == 1 and k1 >= 40:
                            nc.sync.dma_start(
                                bounceB.ap().rearrange(
                                    "(t p) f -> p t f", p=128),
                                u_next[:, 26:39, :])
                            coll_piece(u_alls[(layer + 1) % 2], 1)
                            collA_issued = 2
                    if layer < 3:
                        nc.sync.dma_start(
                            bounceC.ap().rearrange("(t p) f -> p t f", p=128),
                            u_next[:, 39:NROUND, :])
                        coll_piece(u_alls[(layer + 1) % 2], 2)
                        u_nm = u_next

            # ================= sort-pool + classifier =================
            with tc.tile_pool(name="poolp", bufs=1) as qp:
                h3 = hcatB[64:65, :]
                nc.sync.dma_start(d6656.ap(), h3)
                h3gt = qp.tile([13, NPG], f32)
                nc.sync.dma_start(
                    h3gt[:],
                    d6656.ap()[:, 0:6500].rearrange(
                        "one (i g) -> (one g) i", g=13))
                h3g = h3gt[:]

                m8a = qp.tile([13, 8], f32)
                i8a = qp.tile([13, 8], mybir.dt.uint32)
                nc.vector.max(m8a[:], h3g)
                nc.vector.max_index(i8a[:], m8a[:], h3g)
                h3m = qp.tile([13, NPG], f32)
                nc.vector.match_replace(h3m[:], m8a[:], h3g, -2.0)
                m8b = qp.tile([13, 8], f32)
                i8b = qp.tile([13, 8], mybir.dt.uint32)
                nc.vector.max(m8b[:], h3m[:])
                nc.vector.max_index(i8b[:], m8b[:], h3m[:])

                idx2d = qp.tile([13, 16], f32)
                nc.vector.memset(idx2d[:], 0.0)
                nc.vector.tensor_copy(idx2d[:, 0:8], i8a[:])
                nc.vector.tensor_copy(idx2d[:, 8:15], i8b[:, 0:7])
                gof = qp.tile([13, 1], mybir.dt.int32)
                nc.gpsimd.iota(gof[:], [[0, 1]], base=0, channel_multiplier=1)
                goff = qp.tile([13, 1], f32)
                nc.vector.tensor_copy(goff[:], gof[:])
                # absolute slot = pos*13 + g
                nc.vector.tensor_scalar(
                    out=idx2d[:], in0=idx2d[:], scalar1=13.0,
                    scalar2=None, op0=mybir.AluOpType.mult)
                nc.vector.tensor_scalar(
                    out=idx2d[:], in0=idx2d[:], scalar1=goff[:, 0:1],
                    scalar2=None, op0=mybir.AluOpType.add)
                idx16 = qp.tile([13, 16], mybir.dt.int16)
                nc.vector.tensor_copy(idx16[:], idx2d[:])
                nc.sync.dma_start(
                    d208.ap().rearrange("one (g r) -> (one g) r", g=13),
                    idx16[:])
                idx128 = qp.tile([128, 13], mybir.dt.int16)
                for kk in range(8):
                    nc.sync.dma_start(
                        idx128[kk * 16:(kk + 1) * 16, :],
                        d208.ap().rearrange("one (s p) -> (one p) s", p=16))

                poolA = qp.tile([128, 208], f32)
                nc.gpsimd.ap_gather(poolA[:], hcatA[:], idx128[:],
                                    channels=128, num_elems=SHARD, d=1,
                                    num_idxs=208)
                poolB = qp.tile([80, 208], f32)
                nc.gpsimd.ap_gather(poolB[:], hcatB[:], idx128[0:80, :],
                                    channels=80, num_elems=SHARD, d=1,
                                    num_idxs=208)

                z1 = []
                for mh in range(2):
                    pz = ps_w.tile([128, 13], f32, tag="pw", name=f"pz{mh}")
                    first = True
                    for r in range(K_TOP):
                        nc.tensor.matmul(
                            pz[:], WA[:, r, mh * 128:(mh + 1) * 128],
                            poolA[:, r:r + 16 * 12 + 1:16],
                            start=first, stop=False)
                        first = False
                        nc.tensor.matmul(
                            pz[:], WB[0:65, r, mh * 128:(mh + 1) * 128],
                            poolB[0:65, r:r + 16 * 12 + 1:16],
                            start=False, stop=(r == K_TOP - 1))
                    zz = qp.tile([128, 13], f32, tag=f"z1_{mh}", name=f"z1_{mh}")
                    nc.scalar.activation(zz[:], pz[:], AF.Relu,
                                         bias=be[:, mh:mh + 1],
                                         scale=sc[:, mh:mh + 1])
                    z1.append(zz)
                pz2 = ps_w.tile([128, 13], f32, tag="pw", name="pz2")
                nc.tensor.matmul(pz2[:], Wc1s[:, 0, :], z1[0][:],
                                 start=True, stop=False)
                nc.tensor.matmul(pz2[:], Wc1s[:, 1, :], z1[1][:],
                                 start=False, stop=True)
                z2 = qp.tile([128, 13], f32)
                nc.scalar.activation(z2[:], pz2[:], AF.Relu, bias=bc1[:])
                pz3 = ps_w.tile([F, 13], f32, tag="pw", name="pz3")
                nc.tensor.matmul(pz3[:], Wc2[:], z2[:], start=True, stop=True)
                z3 = qp.tile([F, 13], f32)
                nc.scalar.activation(z3[:], pz3[:], AF.Relu, bias=bc2[:])
                pz4 = ps_w.tile([1, 13], f32, tag="pw", name="pz4")
                nc.tensor.matmul(pz4[:], Wc3[:], z3[:], start=True, stop=True)
                zf = qp.tile([1, 13], f32)
                nc.vector.tensor_scalar(out=zf[:], in0=pz4[:],
                                        scalar1=bc3[0:1, 0:1], scalar2=None,
                                        op0=mybir.AluOpType.add)
                nc.sync.dma_start(out_d[:], zf[:])

    nc.compile()
    return nc


def kernel(**inputs):
    from concourse import bass_utils

    x = np.asarray(inputs["x"], np.float32)
    edge_index = np.asarray(inputs["edge_index"])

    key = ("prog",)
    CALLS, D, callbase, gidxs, u0bs, dinv_nms, u0_full = _prep(x, edge_index)
    if key in _CACHE and _CACHE[key][0] == CALLS and np.array_equal(_CACHE[key][1], D):
        nc = _CACHE[key][2]
    else:
        nc = _build(CALLS, D, callbase)
        _CACHE[key] = (CALLS, D, nc)

    cw = _pack_classifier(inputs)
    in_maps = []
    for c in range(NCORES):
        m = {
            "u0b": u0bs[c],
            "u0_full": u0_full,
            "gidx": gidxs[c],
            "dinv_nm": dinv_nms[c],
            "WA": cw["WA"], "WB": cw["WB"], "sc": cw["sc"], "be": cw["be"],
            "Wc1s": cw["Wc1s"], "bc1": cw["bc1"], "Wc2": cw["Wc2"],
            "bc2": cw["bc2"], "Wc3": cw["Wc3"], "bc3": cw["bc3"],
        }
        for i in range(4):
            m[f"W{i}"] = np.asarray(inputs[f"W{i}"], np.float32).reshape(
                F, F if i < 3 else 1)
            m[f"b{i}"] = np.asarray(inputs[f"b{i}"], np.float32).reshape(
                F if i < 3 else 1, 1)
        in_maps.append(m)

    trace = os.environ.get("KERNEL_TRACE", "0") == "1"
    kwargs = {}
    if trace:
        import sys, types
        if "antenv.axon_hooks" not in sys.modules:
            sys.path.insert(0, "/root/.axon_site")
            from trn_agent_boot.trn_boot import _ntff_profile_via_ctypes
            mm = types.ModuleType("antenv.axon_hooks")
            mm.get_axon_ntff_profile_hook = (
                lambda: _ntff_profile_via_ctypes("/opt/axon/libaxon_pjrt.so"))
            sys.modules["antenv.axon_hooks"] = mm
        import tempfile
        kwargs = dict(trace=True, tmpdir=tempfile.mkdtemp())

    res = bass_utils.run_bass_kernel_spmd(
        nc, in_maps, core_ids=list(range(NCORES)), **kwargs)

    global LAST_EXEC_NS
    LAST_EXEC_NS = res.exec_time_ns

    out = np.zeros((G, 1), np.float32)
    for c in range(NCORES):
        ngr = GRAPHS_PER_CORE[c]
        out[GSTART[c]:GSTART[c] + ngr, 0] = res.results[c]["out"][0, :ngr]
    return out


LAST_EXEC_NS = None
